# revision 10
# baseline (speedup 1.0000x reference)
"""TRN2 Bass kernel for nn_DecoderLayer: masked self-attention + cross-attention
+ 2-layer ReLU FFN, data-parallel over the batch dim across 8 NeuronCores.

Contract: kernel(**inputs) takes FULL unsharded inputs (numpy arrays, keyed as
in reference.setup_inputs()) and returns the FULL [8, 2048, 512] fp32 output.

Per-core computation (one batch element b):
    attn1 = softmax(y_b @ y_b.T / sqrt(D) masked) @ y_b
    attn2 = softmax(attn1 @ enc_b.T / sqrt(D)) @ enc_b
    out_b = relu(attn2 @ W1 + b1) @ W2 + b2

The mask is all-ones for this problem's input distribution (spec fill=ones);
the device kernel assumes that and the host wrapper verifies it, falling back
to a numpy reference in the (never exercised) general-mask case.

Kernel strategy ("transposed flash", v3): activations stay in transposed
layout [d, seq] so probability tiles never need transposing.  Scores are
computed in [k, q] layout, exp on ACT without max-subtraction (scores bounded
for these inputs), softmax denominators accumulated as a bf16 pairwise tree
on DVE (the per-k-tile ones-matmuls of v1 cost a full 512-column PE slot
each, 10% of all PE work; the final partition reduction is one ones-matmul
per block, and the PE sums partitions exactly in f32 so the bf16 partials
cost ~0.04% denominator error).

Empirical PE model from the v1/v2 traces: every matmul issues at
~max(N_out x 0.42ns, LDWEIGHTS + 40ns) regardless of dtype; fp8-DoubleRow
does NOT stream columns faster, it only halves instruction count (K=256 per
instruction), and a 4-byte f32/f32r stationary LDWEIGHTS (189ns) gates the
213ns column stream.  The walrus verifier also rejects mixing f32/f32r with
other dtypes in one matmul.  Hence: self-attention scores run fp8-e4m3
DoubleRow (noise suppressed by the near-identity softmax), and every other
matmul runs bf16 x bf16 (LDWEIGHTS hides, stream-bound at ~216ns/matmul,
~4e-3 output error vs the 2e-2 gate).

Scheduling: input DMA is chunked and pipelined into the first self-attention
block (PE starts ~12us in, bounded by the engine preamble + first chunk).
Transposes read the persistent bf16 copies, write paired [128, 2x2x128] PSUM
generations, and are drained one generation per k-tile group through a
filler queue so the single PSUM bank never stalls the PE.  Each block's
epilogue is split and deferred into the NEXT block: the DVE tree-tail folds
flush after k-tile 0, the denominator matmul + normalization after k-tile 3,
so the PE never waits on the exp/esum tail or the PSUM-release copies.
"""

import numpy as np

B, SD, SE, D = 8, 2048, 1024, 512
P = 128
N_CORES = 8

_CACHE = {}
LAST_RESULT = None


def _install_ntff_shim():
    """Provide antenv.axon_hooks if the image lacks it, so that
    run_bass_kernel_spmd(trace=True) (BASS_TRACE=1) can capture NTFF
    profiles via libaxon's C ABI instead of crashing on the import."""
    import sys
    try:
        import antenv.axon_hooks  # noqa: F401
        return
    except ImportError:
        pass
    import contextlib
    import ctypes
    import types

    _hook = [None]
    so = "/opt/axon/libaxon_pjrt.so"
    try:
        lib = ctypes.CDLL(so)
        if hasattr(lib, "axon_start_nrt_profile"):
            lib.axon_start_nrt_profile.argtypes = [
                ctypes.POINTER(ctypes.c_int64), ctypes.c_size_t]
            lib.axon_start_nrt_profile.restype = ctypes.c_int64
            lib.axon_stop_nrt_profile.argtypes = [ctypes.c_char_p]
            lib.axon_stop_nrt_profile.restype = ctypes.c_int64

            @contextlib.contextmanager
            def hook(output_dir, device_ids):
                import jax
                jax.devices()
                if device_ids:
                    ids = (ctypes.c_int64 * len(device_ids))(*device_ids)
                    rc = lib.axon_start_nrt_profile(ids, len(device_ids))
                else:
                    rc = lib.axon_start_nrt_profile(None, 0)
                if rc != 0:
                    raise RuntimeError(f"axon_start_nrt_profile rc={rc}")
                try:
                    yield
                finally:
                    n = lib.axon_stop_nrt_profile(str(output_dir).encode())
                    if n <= 0:
                        import sys as _s
                        print(f"ntff profile: {n} files written", file=_s.stderr)

            _hook[0] = hook
    except OSError:
        pass

    mod = types.ModuleType("antenv.axon_hooks")
    mod.get_axon_ntff_profile_hook = lambda: _hook[0]

    def _set(h):
        _hook[0] = h

    mod.set_axon_ntff_profile_hook = _set
    import antenv
    antenv.axon_hooks = mod
    sys.modules["antenv.axon_hooks"] = mod


try:
    _install_ntff_shim()
except Exception:
    pass


def _build_module(sd=SD, se=SE, qb=512):
    import concourse.tile as tile
    from concourse import bacc, mybir
    from concourse.masks import make_identity

    FP32 = mybir.dt.float32
    BF16 = mybir.dt.bfloat16
    F8 = mybir.dt.float8e4
    Act = mybir.ActivationFunctionType
    DR = mybir.MatmulPerfMode.DoubleRow

    DC = D // P           # d chunks (4)
    NQB = sd // qb        # num q blocks (4)
    KT1 = sd // P         # stage-1 k tiles (16)
    KT2 = se // P         # stage-2 k tiles (8)
    QT = qb // P          # q tiles per block (4)
    YC = 2                # 128-row tiles per DMA chunk
    NYC = KT1 // YC       # num y chunks (8)
    scale = 1.0 / float(np.sqrt(D))

    nc = bacc.Bacc("TRN2", target_bir_lowering=False, debug=False,
                   enable_asserts=False, num_devices=N_CORES)
    y_d = nc.dram_tensor("y", (sd, D), FP32, kind="ExternalInput").ap()
    enc_d = nc.dram_tensor("enc", (se, D), FP32, kind="ExternalInput").ap()
    w1_d = nc.dram_tensor("w1", (D, D), FP32, kind="ExternalInput").ap()
    b1_d = nc.dram_tensor("b1", (D,), FP32, kind="ExternalInput").ap()
    w2_d = nc.dram_tensor("w2", (D, D), FP32, kind="ExternalInput").ap()
    b2_d = nc.dram_tensor("b2", (D,), FP32, kind="ExternalInput").ap()
    out_d = nc.dram_tensor("out", (sd, D), FP32, kind="ExternalOutput").ap()

    with tile.TileContext(nc) as tc, \
            tc.tile_pool(name="persist", bufs=1) as persist, \
            tc.tile_pool(name="stage", bufs=4) as stage, \
            tc.tile_pool(name="work", bufs=2) as work, \
            tc.tile_pool(name="blk", bufs=2) as blk, \
            tc.tile_pool(name="psum", bufs=1, space="PSUM") as psum, \
            tc.tile_pool(name="psmm", bufs=2, space="PSUM") as psmm, \
            tc.tile_pool(name="pss", bufs=1, space="PSUM") as pss:

        ident_b = persist.tile([P, P], BF16, tag="ident_b")
        make_identity(nc, ident_b[:])
        ones_f32 = persist.tile([P, 1], FP32, tag="ones_f32")
        nc.gpsimd.memset(ones_f32[:], 1.0)
        ones_b = persist.tile([P, 1], BF16, tag="ones_b")
        nc.vector.tensor_copy(ones_b[:], ones_f32[:])

        # persistent device-resident operands (bf16 except fp8 score copies)
        y_v = persist.tile([P, KT1, D], BF16, tag="y_v")       # V for stage 1
        yT8 = persist.tile([P, DC, sd], F8, tag="yT8")         # Q/K for stage 1
        enc_v = persist.tile([P, KT2, D], BF16, tag="enc_v")   # V for stage 2
        encT = persist.tile([P, DC, se], BF16, tag="encT")     # K^T for stage 2
        w1_sb = persist.tile([P, DC, D], BF16, tag="w1_sb")    # FFN1 stationary
        w2_sb = persist.tile([P, DC, D], BF16, tag="w2_sb")    # FFN2 moving
        b1_sb = persist.tile([P, DC], FP32, tag="b1_sb")
        b2_sb = persist.tile([P, D], FP32, tag="b2_sb")
        attn1T = persist.tile([P, DC, sd], BF16, tag="attn1T")
        attn2T = persist.tile([P, DC, sd], BF16, tag="attn2T")

        # ---- pipelined input staging -------------------------------------
        def load_chunk(src_rows):
            """DMA 2x128 rows of a [*, 512] f32 DRAM tensor into staging."""
            stg = stage.tile([P, YC, D], FP32, tag="stg")
            nc.sync.dma_start(stg[:],
                              src_rows.rearrange("(t p) c -> p t c", p=P))
            return stg

        # filler queue: each entry emits one PSUM transpose generation (4
        # transposes + 2 batched copies); drained one per k-tile group so
        # the single tp PSUM bank never stalls the PE.
        fillers = []

        def drain_filler():
            if fillers:
                fillers.pop(0)()

        def t_gen(src_v, dstT, st0, h):
            """Transpose dc pair (2h, 2h+1) of tiles (st0, st0+1) into dstT."""
            tp = psmm.tile([P, 2, YC, P], BF16, tag="tp", bufs=1, name="tp")
            for i in range(2):
                dc = 2 * h + i
                for t in range(YC):
                    nc.tensor.transpose(
                        tp[:, i, t, :],
                        src_v[:, st0 + t, dc * P:(dc + 1) * P], ident_b[:])
                nc.vector.tensor_copy(dstT[:, dc, st0 * P:(st0 + YC) * P],
                                      tp[:, i, :, :])

        # ---- deferred block epilogue --------------------------------------
        # stage_a (after next block's k-tile 0): DVE folds of the esum tree
        # leftovers; stage_b (after k-tile 3): denominator matmul + normalize.
        pending = []

        def stage_a(ent):
            accs, leftovers, outT_b = ent
            s = leftovers[0]
            for t in leftovers[1:]:
                f = work.tile([P, qb], BF16, tag="fold", bufs=2, name="fold")
                nc.gpsimd.tensor_add(f[:], s[:], t[:])
                s = f
            return (accs, s, outT_b)

        def stage_b(ent):
            accs, esum, outT_b = ent
            dn = pss.tile([1, qb], FP32, tag="dn")
            nc.tensor.matmul(dn[:], ones_b[:], esum[:], start=True, stop=True)
            rrow = work.tile([1, qb], FP32, tag="rrow", bufs=2)
            nc.vector.reciprocal_approx_fast(rrow[:], dn[:])
            rbc = work.tile([P, qb], FP32, tag="rbc", bufs=2)
            nc.gpsimd.partition_broadcast(rbc[:], rrow[:])
            for dc in range(DC):
                nc.vector.tensor_mul(outT_b[:, dc, :], accs[dc][:], rbc[:])

        def epilogue_hooks(kt):
            if kt == 0 and pending:
                pending[0] = stage_a(pending[0])
            elif kt == 3 and pending:
                stage_b(pending.pop(0))
            drain_filler()

        # ---- one attention block -------------------------------------------
        def attn_block(kt_n, emit_scores, v_sb, tag):
            """Scores+exp+attn@V+esum-tree for one q block.  Returns SBUF
            copies of the accumulators and the un-folded tree leftovers."""
            acc = [psum.tile([P, qb], FP32, tag=f"acc{dc}", name=f"acc{dc}")
                   for dc in range(DC)]
            lvl = [[] for _ in range(6)]

            def tree_push(t, i=0):
                lvl[i].append(t)
                if len(lvl[i]) == 2:
                    a, b_ = lvl[i]
                    lvl[i].clear()
                    s = work.tile([P, qb], BF16, tag=f"ts{tag}_{i}", bufs=2,
                                  name="tsum")
                    nc.gpsimd.tensor_add(s[:], a[:], b_[:])
                    tree_push(s, i + 1)

            def emit_sc(kt):
                sc = psmm.tile([P, qb], FP32, tag="mm", name="sc")
                emit_scores(sc, kt)
                return sc

            leftovers = []
            sc_next = emit_sc(0)
            for kt in range(kt_n):
                sc_cur, sc_next = sc_next, (emit_sc(kt + 1)
                                            if kt + 1 < kt_n else None)
                e = work.tile([P, qb], BF16, tag=f"e{tag}", bufs=4)
                nc.scalar.activation(e[:], sc_cur[:], Act.Exp, scale=scale)
                for dc in range(DC):
                    nc.tensor.matmul(
                        acc[dc][:], v_sb[:, kt, dc * P:(dc + 1) * P], e[:],
                        start=(kt == 0), stop=(kt == kt_n - 1),
                    )
                if kt < kt_n - 1:
                    tree_push(e)
                else:
                    leftovers = [e] + [l[0] for l in lvl if l]
                epilogue_hooks(kt)
            accs = [work.tile([P, qb], FP32, tag=f"as{tag}", bufs=4,
                              name=f"accs{dc}") for dc in range(DC)]
            for dc in range(DC):
                nc.scalar.copy(accs[dc][:], acc[dc][:])
            return accs, leftovers

        def s1_scores(sc, kt, qc):
            for dh in range(DC // 2):
                nc.tensor.matmul(
                    sc[:], yT8[:, 2 * dh:2 * dh + 2, kt * P:(kt + 1) * P],
                    yT8[:, 2 * dh:2 * dh + 2, qc],
                    start=(dh == 0), stop=(dh == DC // 2 - 1),
                    perf_mode=DR,
                )

        def s2_scores(sc, kt, qc):
            for dc in range(DC):
                nc.tensor.matmul(
                    sc[:], encT[:, dc, kt * P:(kt + 1) * P],
                    attn1T[:, dc, qc],
                    start=(dc == 0), stop=(dc == DC - 1),
                )

        # ==== stage 1 block 0, pipelined with the y input DMA ==============
        # k-tile group {2c, 2c+1} needs y chunk c; the q side (moving fp8)
        # needs chunks 0-1 up front.  DMA runs ~2 chunks ahead of the PE.
        qc0 = slice(0, qb)
        stg0 = load_chunk(y_d[0:YC * P, :])
        stg1 = load_chunk(y_d[YC * P:2 * YC * P, :])
        pend = load_chunk(y_d[2 * YC * P:3 * YC * P, :])
        pend2 = load_chunk(y_d[3 * YC * P:4 * YC * P, :])
        nc.vector.tensor_copy(y_v[:, 0:YC, :], stg0[:])
        nc.vector.tensor_copy(y_v[:, YC:2 * YC, :], stg1[:])
        for c in range(2):
            for h in range(2):
                t_gen(y_v, yT8, c * YC, h)

        acc0 = [psum.tile([P, qb], FP32, tag=f"acc{dc}", name=f"acc{dc}")
                for dc in range(DC)]
        lvl0 = [[] for _ in range(6)]

        def tree_push0(t, i=0):
            lvl0[i].append(t)
            if len(lvl0[i]) == 2:
                a, b_ = lvl0[i]
                lvl0[i].clear()
                s = work.tile([P, qb], BF16, tag=f"ts1_{i}", bufs=2,
                              name="tsum")
                nc.gpsimd.tensor_add(s[:], a[:], b_[:])
                tree_push0(s, i + 1)

        leftovers0 = []
        sc_next = psmm.tile([P, qb], FP32, tag="mm", name="sc")
        s1_scores(sc_next, 0, qc0)
        for kt in range(KT1):
            if kt % YC == 1:
                c = (kt + 3) // YC  # next chunk to prepare
                if c < NYC:
                    nc.vector.tensor_copy(y_v[:, c * YC:(c + 1) * YC, :],
                                          pend[:])
                    fillers.append(lambda c=c: t_gen(y_v, yT8, c * YC, 0))
                    fillers.append(lambda c=c: t_gen(y_v, yT8, c * YC, 1))
                    pend, pend2 = pend2, (
                        load_chunk(y_d[(c+2)*YC*P:(c+3)*YC*P, :])
                        if c + 2 < NYC else None)
            sc_cur = sc_next
            if kt + 1 < KT1:
                sc_next = psmm.tile([P, qb], FP32, tag="mm", name="sc")
                s1_scores(sc_next, kt + 1, qc0)
            else:
                sc_next = None
            e = work.tile([P, qb], BF16, tag="e1", bufs=4)
            nc.scalar.activation(e[:], sc_cur[:], Act.Exp, scale=scale)
            for dc in range(DC):
                nc.tensor.matmul(
                    acc0[dc][:], y_v[:, kt, dc * P:(dc + 1) * P], e[:],
                    start=(kt == 0), stop=(kt == KT1 - 1),
                )
            if kt < KT1 - 1:
                tree_push0(e)
            else:
                leftovers0 = [e] + [l[0] for l in lvl0 if l]
            drain_filler()
        accs0 = [work.tile([P, qb], FP32, tag="as1", bufs=4,
                           name=f"accs{dc}") for dc in range(DC)]
        for dc in range(DC):
            nc.scalar.copy(accs0[dc][:], acc0[dc][:])
        pending.append((accs0, leftovers0, attn1T[:, :, qc0]))

        # remaining inputs: DMA + immediate bf16 casts (releases staging);
        # the enc transposes become fillers drained inside stage-1 block 1.
        for c in range(KT2 // YC):
            stg = load_chunk(enc_d[c * YC * P:(c + 1) * YC * P, :])
            nc.vector.tensor_copy(enc_v[:, c * YC:(c + 1) * YC, :], stg[:])
            fillers.append(lambda c=c: t_gen(enc_v, encT, c * YC, 0))
            fillers.append(lambda c=c: t_gen(enc_v, encT, c * YC, 1))
        for w_sb, w_src in ((w1_sb, w1_d), (w2_sb, w2_d)):
            for c in range(DC // YC):
                stg = load_chunk(w_src[c * YC * P:(c + 1) * YC * P, :])
                nc.vector.tensor_copy(w_sb[:, c * YC:(c + 1) * YC, :], stg[:])
        nc.sync.dma_start(b1_sb[:], b1_d.rearrange("(c p) -> p c", p=P))
        nc.sync.dma_start(b2_sb[:], b2_d.partition_broadcast(P))

        # ==== stage 1 blocks 1-3 ===========================================
        for b in range(1, NQB):
            qc = slice(b * qb, (b + 1) * qb)
            accs, leftovers = attn_block(
                KT1, lambda sc, kt, qc=qc: s1_scores(sc, kt, qc), y_v, "1")
            pending.append((accs, leftovers, attn1T[:, :, qc]))

        # ==== stage 2 ======================================================
        for b in range(NQB):
            qc = slice(b * qb, (b + 1) * qb)
            accs, leftovers = attn_block(
                KT2, lambda sc, kt, qc=qc: s2_scores(sc, kt, qc), enc_v, "2")
            pending.append((accs, leftovers, attn2T[:, :, qc]))

        # ==== FFN (ffn1/ffn2 staggered one block apart) ====================
        def ffn1(b, hooks=False):
            qc = slice(b * qb, (b + 1) * qb)
            hb = blk.tile([P, DC, qb], BF16, tag="hb", bufs=3, name="hb")
            for oc in range(DC):
                hp = psmm.tile([P, qb], FP32, tag="mm", name="hp")
                for ic in range(DC):
                    nc.tensor.matmul(hp[:], w1_sb[:, ic, oc * P:(oc + 1) * P],
                                     attn2T[:, ic, qc],
                                     start=(ic == 0), stop=(ic == DC - 1))
                nc.scalar.activation(hb[:, oc, :], hp[:], Act.Relu,
                                     bias=b1_sb[:, oc:oc + 1])
                if hooks and oc == 0 and pending:
                    pending[0] = stage_a(pending[0])
                if hooks and oc == 3 and pending:
                    stage_b(pending.pop(0))
            return hb

        def ffn2(b, hb):
            for qt in range(QT):
                q0 = b * qb + qt * P
                op = psmm.tile([P, D], FP32, tag="mm", name="op")
                for ic in range(DC):
                    nc.tensor.matmul(op[:], hb[:, ic, qt * P:(qt + 1) * P],
                                     w2_sb[:, ic, :],
                                     start=(ic == 0), stop=(ic == DC - 1))
                ob = work.tile([P, D], FP32, tag="ob", bufs=4)
                nc.vector.tensor_add(ob[:], op[:], b2_sb[:])
                nc.sync.dma_start(out_d[q0:q0 + P, :], ob[:])

        hb_prev = ffn1(0, hooks=True)
        for b in range(1, NQB):
            hb_cur = ffn1(b)
            ffn2(b - 1, hb_prev)
            hb_prev = hb_cur
        ffn2(NQB - 1, hb_prev)

    nc.compile()
    return nc


def _get_module():
    if "mod" not in _CACHE:
        _CACHE["mod"] = _build_module()
    return _CACHE["mod"]


def _reference_fallback(y, encoder_output, mask, W1, b1, W2, b2):
    """General-mask numpy fallback (not exercised for the spec inputs)."""
    NEG_INF = -1e9

    def sdpa(q, k, v, m):
        s = (q @ k.transpose(0, 2, 1)) / np.float32(np.sqrt(q.shape[-1]))
        if m is not None:
            s = np.where(m, s, NEG_INF)
        s = s - s.max(axis=-1, keepdims=True)
        e = np.exp(s)
        p = e / e.sum(axis=-1, keepdims=True)
        return p @ v

    a1 = sdpa(y, y, y, mask)
    a2 = sdpa(a1, encoder_output, encoder_output, None)
    h = np.maximum(a2 @ W1 + b1, 0.0)
    return (h @ W2 + b2).astype(np.float32)


def kernel(y, encoder_output, mask, W1, b1, W2, b2):
    global LAST_RESULT
    y = np.ascontiguousarray(np.asarray(y, dtype=np.float32))
    enc = np.ascontiguousarray(np.asarray(encoder_output, dtype=np.float32))
    W1 = np.ascontiguousarray(np.asarray(W1, dtype=np.float32))
    b1 = np.ascontiguousarray(np.asarray(b1, dtype=np.float32))
    W2 = np.ascontiguousarray(np.asarray(W2, dtype=np.float32))
    b2 = np.ascontiguousarray(np.asarray(b2, dtype=np.float32))

    if mask is not None and not np.asarray(mask).all():
        return _reference_fallback(y, enc, np.asarray(mask), W1, b1, W2, b2)

    from concourse import bass_utils

    nc = _get_module()
    in_maps = [
        {"y": y[i], "enc": enc[i], "w1": W1, "b1": b1, "w2": W2, "b2": b2}
        for i in range(N_CORES)
    ]
    res = bass_utils.run_bass_kernel_spmd(nc, in_maps, core_ids=list(range(N_CORES)))
    LAST_RESULT = res
    return np.stack([res.results[i]["out"] for i in range(N_CORES)], axis=0)


# revision 11
# speedup vs baseline: 1.2933x; 1.2933x over previous
"""TRN2 Bass kernel for nn_DecoderLayer: masked self-attention + cross-attention
+ 2-layer ReLU FFN, data-parallel over the batch dim across 8 NeuronCores.

Contract: kernel(**inputs) takes FULL unsharded inputs (numpy arrays, keyed as
in reference.setup_inputs()) and returns the FULL [8, 2048, 512] fp32 output.

Per-core computation (one batch element b):
    attn1 = softmax(y_b @ y_b.T / sqrt(D) masked) @ y_b
    attn2 = softmax(attn1 @ enc_b.T / sqrt(D)) @ enc_b
    out_b = relu(attn2 @ W1 + b1) @ W2 + b2

The mask is all-ones for this problem's input distribution (spec fill=ones);
the device kernel assumes that and the host wrapper verifies it, falling back
to a numpy reference in the (never exercised) general-mask case.

Kernel strategy ("transposed flash", v3): activations stay in transposed
layout [d, seq] so probability tiles never need transposing.  Scores are
computed in [k, q] layout, exp on ACT without max-subtraction (scores bounded
for these inputs), softmax denominators accumulated as a bf16 pairwise tree
on DVE (the per-k-tile ones-matmuls of v1 cost a full 512-column PE slot
each, 10% of all PE work; the final partition reduction is one ones-matmul
per block, and the PE sums partitions exactly in f32 so the bf16 partials
cost ~0.04% denominator error).

Empirical PE model from the v1/v2 traces: every matmul issues at
~max(N_out x 0.42ns, LDWEIGHTS + 40ns) regardless of dtype; fp8-DoubleRow
does NOT stream columns faster, it only halves instruction count (K=256 per
instruction), and a 4-byte f32/f32r stationary LDWEIGHTS (189ns) gates the
213ns column stream.  The walrus verifier also rejects mixing f32/f32r with
other dtypes in one matmul.  Hence: self-attention scores run fp8-e4m3
DoubleRow (noise suppressed by the near-identity softmax), and every other
matmul runs bf16 x bf16 (LDWEIGHTS hides, stream-bound at ~216ns/matmul,
~4e-3 output error vs the 2e-2 gate).

Scheduling: input DMA is chunked and pipelined into the first self-attention
block (PE starts ~12us in, bounded by the engine preamble + first chunk).
Transposes read the persistent bf16 copies, write paired [128, 2x2x128] PSUM
generations, and are drained one generation per k-tile group through a
filler queue so the single PSUM bank never stalls the PE.  Each block's
epilogue is split and deferred into the NEXT block: the DVE tree-tail folds
flush after k-tile 0, the denominator matmul + normalization after k-tile 3,
so the PE never waits on the exp/esum tail or the PSUM-release copies.
"""

import numpy as np

B, SD, SE, D = 8, 2048, 1024, 512
P = 128
N_CORES = 8

_CACHE = {}
LAST_RESULT = None


def _install_ntff_shim():
    """Provide antenv.axon_hooks if the image lacks it, so that
    run_bass_kernel_spmd(trace=True) (BASS_TRACE=1) can capture NTFF
    profiles via libaxon's C ABI instead of crashing on the import."""
    import sys
    try:
        import antenv.axon_hooks  # noqa: F401
        return
    except ImportError:
        pass
    import contextlib
    import ctypes
    import types

    _hook = [None]
    so = "/opt/axon/libaxon_pjrt.so"
    try:
        lib = ctypes.CDLL(so)
        if hasattr(lib, "axon_start_nrt_profile"):
            lib.axon_start_nrt_profile.argtypes = [
                ctypes.POINTER(ctypes.c_int64), ctypes.c_size_t]
            lib.axon_start_nrt_profile.restype = ctypes.c_int64
            lib.axon_stop_nrt_profile.argtypes = [ctypes.c_char_p]
            lib.axon_stop_nrt_profile.restype = ctypes.c_int64

            @contextlib.contextmanager
            def hook(output_dir, device_ids):
                import jax
                jax.devices()
                if device_ids:
                    ids = (ctypes.c_int64 * len(device_ids))(*device_ids)
                    rc = lib.axon_start_nrt_profile(ids, len(device_ids))
                else:
                    rc = lib.axon_start_nrt_profile(None, 0)
                if rc != 0:
                    raise RuntimeError(f"axon_start_nrt_profile rc={rc}")
                try:
                    yield
                finally:
                    n = lib.axon_stop_nrt_profile(str(output_dir).encode())
                    if n <= 0:
                        import sys as _s
                        print(f"ntff profile: {n} files written", file=_s.stderr)

            _hook[0] = hook
    except OSError:
        pass

    mod = types.ModuleType("antenv.axon_hooks")
    mod.get_axon_ntff_profile_hook = lambda: _hook[0]

    def _set(h):
        _hook[0] = h

    mod.set_axon_ntff_profile_hook = _set
    import antenv
    antenv.axon_hooks = mod
    sys.modules["antenv.axon_hooks"] = mod


try:
    _install_ntff_shim()
except Exception:
    pass


def _build_module(sd=SD, se=SE, qb=512):
    import concourse.tile as tile
    from concourse import bacc, mybir
    from concourse.masks import make_identity

    FP32 = mybir.dt.float32
    BF16 = mybir.dt.bfloat16
    F8 = mybir.dt.float8e4
    Act = mybir.ActivationFunctionType
    DR = mybir.MatmulPerfMode.DoubleRow

    DC = D // P           # d chunks (4)
    NQB = sd // qb        # num q blocks (4)
    KT1 = sd // P         # stage-1 k tiles (16)
    KT2 = se // P         # stage-2 k tiles (8)
    QT = qb // P          # q tiles per block (4)
    YC = 2                # 128-row tiles per DMA chunk
    NYC = KT1 // YC       # num y chunks (8)
    scale = 1.0 / float(np.sqrt(D))

    nc = bacc.Bacc("TRN2", target_bir_lowering=False, debug=False,
                   enable_asserts=False, num_devices=N_CORES)
    y_d = nc.dram_tensor("y", (sd, D), FP32, kind="ExternalInput").ap()
    enc_d = nc.dram_tensor("enc", (se, D), FP32, kind="ExternalInput").ap()
    w1_d = nc.dram_tensor("w1", (D, D), FP32, kind="ExternalInput").ap()
    b1_d = nc.dram_tensor("b1", (D,), FP32, kind="ExternalInput").ap()
    w2_d = nc.dram_tensor("w2", (D, D), FP32, kind="ExternalInput").ap()
    b2_d = nc.dram_tensor("b2", (D,), FP32, kind="ExternalInput").ap()
    out_d = nc.dram_tensor("out", (sd, D), FP32, kind="ExternalOutput").ap()

    with tile.TileContext(nc) as tc, \
            tc.tile_pool(name="persist", bufs=1) as persist, \
            tc.tile_pool(name="stage", bufs=4) as stage, \
            tc.tile_pool(name="work", bufs=2) as work, \
            tc.tile_pool(name="blk", bufs=2) as blk, \
            tc.tile_pool(name="psum", bufs=1, space="PSUM") as psum, \
            tc.tile_pool(name="psmm", bufs=2, space="PSUM") as psmm, \
            tc.tile_pool(name="pss", bufs=1, space="PSUM") as pss:

        ident_b = persist.tile([P, P], BF16, tag="ident_b")
        make_identity(nc, ident_b[:])
        ones_f32 = persist.tile([P, 1], FP32, tag="ones_f32")
        nc.gpsimd.memset(ones_f32[:], 1.0)
        ones_b = persist.tile([P, 1], BF16, tag="ones_b")
        nc.vector.tensor_copy(ones_b[:], ones_f32[:])

        # persistent device-resident operands (bf16 except fp8 score copies)
        y_v = persist.tile([P, KT1, D], BF16, tag="y_v")       # V for stage 1
        yT8 = persist.tile([P, DC, sd], F8, tag="yT8")         # Q/K for stage 1
        enc_v = persist.tile([P, KT2, D], BF16, tag="enc_v")   # V for stage 2
        encT = persist.tile([P, DC, se], BF16, tag="encT")     # K^T for stage 2
        w1_sb = persist.tile([P, DC, D], BF16, tag="w1_sb")    # FFN1 stationary
        w2_sb = persist.tile([P, DC, D], BF16, tag="w2_sb")    # FFN2 moving
        b1_sb = persist.tile([P, DC], FP32, tag="b1_sb")
        b2_sb = persist.tile([P, D], FP32, tag="b2_sb")
        attn1T = persist.tile([P, DC, sd], BF16, tag="attn1T")
        attn2T = persist.tile([P, DC, sd], BF16, tag="attn2T")

        # ---- pipelined input staging -------------------------------------
        def load_chunk(src_rows):
            """DMA 2x128 rows of a [*, 512] f32 DRAM tensor into staging."""
            stg = stage.tile([P, YC, D], FP32, tag="stg")
            nc.sync.dma_start(stg[:],
                              src_rows.rearrange("(t p) c -> p t c", p=P))
            return stg

        # filler queue: each entry emits one PSUM transpose generation (4
        # transposes + 2 batched copies); drained one per k-tile group so
        # the single tp PSUM bank never stalls the PE.
        fillers = []

        def drain_filler():
            if fillers:
                fillers.pop(0)()

        def t_gen(src_v, dstT, st0, h):
            """Transpose dc pair (2h, 2h+1) of tiles (st0, st0+1) into dstT."""
            tp = psmm.tile([P, 2, YC, P], BF16, tag="tp", bufs=1, name="tp")
            for i in range(2):
                dc = 2 * h + i
                for t in range(YC):
                    nc.tensor.transpose(
                        tp[:, i, t, :],
                        src_v[:, st0 + t, dc * P:(dc + 1) * P], ident_b[:])
                nc.vector.tensor_copy(dstT[:, dc, st0 * P:(st0 + YC) * P],
                                      tp[:, i, :, :])

        # ---- deferred block epilogue --------------------------------------
        # stage_a (after next block's k-tile 0): DVE folds of the esum tree
        # leftovers; stage_b (after k-tile 3): denominator matmul + normalize.
        pending = []

        def stage_a(ent):
            accs, leftovers, outT_b = ent
            s = leftovers[0]
            for t in leftovers[1:]:
                f = work.tile([P, qb], BF16, tag="fold", bufs=2, name="fold")
                nc.vector.tensor_add(f[:], s[:], t[:])
                s = f
            return (accs, s, outT_b)

        def stage_b(ent):
            accs, esum, outT_b = ent
            dn = pss.tile([1, qb], FP32, tag="dn")
            nc.tensor.matmul(dn[:], ones_b[:], esum[:], start=True, stop=True)
            rrow = work.tile([1, qb], FP32, tag="rrow", bufs=2)
            nc.vector.reciprocal_approx_fast(rrow[:], dn[:])
            rbc = work.tile([P, qb], FP32, tag="rbc", bufs=2)
            nc.gpsimd.partition_broadcast(rbc[:], rrow[:])
            for dc in range(DC):
                nc.gpsimd.tensor_mul(outT_b[:, dc, :], accs[dc][:], rbc[:])

        def epilogue_hooks(kt):
            if kt == 0 and pending:
                pending[0] = stage_a(pending[0])
            elif kt == 3 and pending:
                stage_b(pending.pop(0))
            drain_filler()

        # ---- one attention block -------------------------------------------
        def attn_block(kt_n, emit_scores, v_sb, tag):
            """Scores+exp+attn@V+esum-tree for one q block.  Returns SBUF
            copies of the accumulators and the un-folded tree leftovers."""
            acc = [psum.tile([P, qb], FP32, tag=f"acc{dc}", name=f"acc{dc}")
                   for dc in range(DC)]
            lvl = [[] for _ in range(6)]

            def tree_push(t, i=0):
                lvl[i].append(t)
                if len(lvl[i]) == 2:
                    a, b_ = lvl[i]
                    lvl[i].clear()
                    s = work.tile([P, qb], BF16, tag=f"ts{tag}_{i}", bufs=2,
                                  name="tsum")
                    nc.vector.tensor_add(s[:], a[:], b_[:])
                    tree_push(s, i + 1)

            def emit_sc(kt):
                sc = psmm.tile([P, qb], FP32, tag="mm", name="sc")
                emit_scores(sc, kt)
                return sc

            leftovers = []
            sc_next = emit_sc(0)
            for kt in range(kt_n):
                sc_cur, sc_next = sc_next, (emit_sc(kt + 1)
                                            if kt + 1 < kt_n else None)
                e = work.tile([P, qb], BF16, tag=f"e{tag}", bufs=4)
                nc.scalar.activation(e[:], sc_cur[:], Act.Exp, scale=scale)
                for dc in range(DC):
                    nc.tensor.matmul(
                        acc[dc][:], v_sb[:, kt, dc * P:(dc + 1) * P], e[:],
                        start=(kt == 0), stop=(kt == kt_n - 1),
                    )
                if kt < kt_n - 1:
                    tree_push(e)
                else:
                    leftovers = [e] + [l[0] for l in lvl if l]
                epilogue_hooks(kt)
            accs = [work.tile([P, qb], FP32, tag=f"as{tag}", bufs=4,
                              name=f"accs{dc}") for dc in range(DC)]
            for dc in range(DC):
                nc.vector.tensor_copy(accs[dc][:], acc[dc][:])
            return accs, leftovers

        def s1_scores(sc, kt, qc):
            for dh in range(DC // 2):
                nc.tensor.matmul(
                    sc[:], yT8[:, 2 * dh:2 * dh + 2, kt * P:(kt + 1) * P],
                    yT8[:, 2 * dh:2 * dh + 2, qc],
                    start=(dh == 0), stop=(dh == DC // 2 - 1),
                    perf_mode=DR,
                )

        def s2_scores(sc, kt, qc):
            for dc in range(DC):
                nc.tensor.matmul(
                    sc[:], encT[:, dc, kt * P:(kt + 1) * P],
                    attn1T[:, dc, qc],
                    start=(dc == 0), stop=(dc == DC - 1),
                )

        # ==== stage 1 block 0, pipelined with the y input DMA ==============
        # k-tile group {2c, 2c+1} needs y chunk c; the q side (moving fp8)
        # needs chunks 0-1 up front.  DMA runs ~2 chunks ahead of the PE.
        qc0 = slice(0, qb)
        stg0 = load_chunk(y_d[0:YC * P, :])
        stg1 = load_chunk(y_d[YC * P:2 * YC * P, :])
        pend = load_chunk(y_d[2 * YC * P:3 * YC * P, :])
        pend2 = load_chunk(y_d[3 * YC * P:4 * YC * P, :])
        nc.vector.tensor_copy(y_v[:, 0:YC, :], stg0[:])
        nc.vector.tensor_copy(y_v[:, YC:2 * YC, :], stg1[:])
        for c in range(2):
            for h in range(2):
                t_gen(y_v, yT8, c * YC, h)

        acc0 = [psum.tile([P, qb], FP32, tag=f"acc{dc}", name=f"acc{dc}")
                for dc in range(DC)]
        lvl0 = [[] for _ in range(6)]

        def tree_push0(t, i=0):
            lvl0[i].append(t)
            if len(lvl0[i]) == 2:
                a, b_ = lvl0[i]
                lvl0[i].clear()
                s = work.tile([P, qb], BF16, tag=f"ts1_{i}", bufs=2,
                              name="tsum")
                nc.vector.tensor_add(s[:], a[:], b_[:])
                tree_push0(s, i + 1)

        leftovers0 = []
        sc_next = psmm.tile([P, qb], FP32, tag="mm", name="sc")
        s1_scores(sc_next, 0, qc0)
        for kt in range(KT1):
            if kt % YC == 1:
                c = (kt + 3) // YC  # next chunk to prepare
                if c < NYC:
                    nc.vector.tensor_copy(y_v[:, c * YC:(c + 1) * YC, :],
                                          pend[:])
                    fillers.append(lambda c=c: t_gen(y_v, yT8, c * YC, 0))
                    fillers.append(lambda c=c: t_gen(y_v, yT8, c * YC, 1))
                    pend, pend2 = pend2, (
                        load_chunk(y_d[(c+2)*YC*P:(c+3)*YC*P, :])
                        if c + 2 < NYC else None)
            sc_cur = sc_next
            if kt + 1 < KT1:
                sc_next = psmm.tile([P, qb], FP32, tag="mm", name="sc")
                s1_scores(sc_next, kt + 1, qc0)
            else:
                sc_next = None
            e = work.tile([P, qb], BF16, tag="e1", bufs=4)
            nc.scalar.activation(e[:], sc_cur[:], Act.Exp, scale=scale)
            for dc in range(DC):
                nc.tensor.matmul(
                    acc0[dc][:], y_v[:, kt, dc * P:(dc + 1) * P], e[:],
                    start=(kt == 0), stop=(kt == KT1 - 1),
                )
            if kt < KT1 - 1:
                tree_push0(e)
            else:
                leftovers0 = [e] + [l[0] for l in lvl0 if l]
            drain_filler()
        accs0 = [work.tile([P, qb], FP32, tag="as1", bufs=4,
                           name=f"accs{dc}") for dc in range(DC)]
        for dc in range(DC):
            nc.vector.tensor_copy(accs0[dc][:], acc0[dc][:])
        pending.append((accs0, leftovers0, attn1T[:, :, qc0]))

        # remaining inputs: DMA + immediate bf16 casts (releases staging);
        # the enc transposes become fillers drained inside stage-1 block 1.
        for c in range(KT2 // YC):
            stg = load_chunk(enc_d[c * YC * P:(c + 1) * YC * P, :])
            nc.vector.tensor_copy(enc_v[:, c * YC:(c + 1) * YC, :], stg[:])
            fillers.append(lambda c=c: t_gen(enc_v, encT, c * YC, 0))
            fillers.append(lambda c=c: t_gen(enc_v, encT, c * YC, 1))
        for w_sb, w_src in ((w1_sb, w1_d), (w2_sb, w2_d)):
            for c in range(DC // YC):
                stg = load_chunk(w_src[c * YC * P:(c + 1) * YC * P, :])
                nc.vector.tensor_copy(w_sb[:, c * YC:(c + 1) * YC, :], stg[:])
        nc.sync.dma_start(b1_sb[:], b1_d.rearrange("(c p) -> p c", p=P))
        nc.sync.dma_start(b2_sb[:], b2_d.partition_broadcast(P))

        # ==== stage 1 blocks 1-3 ===========================================
        for b in range(1, NQB):
            qc = slice(b * qb, (b + 1) * qb)
            accs, leftovers = attn_block(
                KT1, lambda sc, kt, qc=qc: s1_scores(sc, kt, qc), y_v, "1")
            pending.append((accs, leftovers, attn1T[:, :, qc]))

        # ==== stage 2 ======================================================
        for b in range(NQB):
            qc = slice(b * qb, (b + 1) * qb)
            accs, leftovers = attn_block(
                KT2, lambda sc, kt, qc=qc: s2_scores(sc, kt, qc), enc_v, "2")
            pending.append((accs, leftovers, attn2T[:, :, qc]))

        # ==== FFN (ffn1/ffn2 staggered one block apart) ====================
        def ffn1(b, hooks=False):
            qc = slice(b * qb, (b + 1) * qb)
            hb = blk.tile([P, DC, qb], BF16, tag="hb", bufs=3, name="hb")
            for oc in range(DC):
                hp = psmm.tile([P, qb], FP32, tag="mm", name="hp")
                for ic in range(DC):
                    nc.tensor.matmul(hp[:], w1_sb[:, ic, oc * P:(oc + 1) * P],
                                     attn2T[:, ic, qc],
                                     start=(ic == 0), stop=(ic == DC - 1))
                nc.scalar.activation(hb[:, oc, :], hp[:], Act.Relu,
                                     bias=b1_sb[:, oc:oc + 1])
                if hooks and oc == 0 and pending:
                    pending[0] = stage_a(pending[0])
                if hooks and oc == 3 and pending:
                    stage_b(pending.pop(0))
            return hb

        def ffn2(b, hb):
            for qt in range(QT):
                q0 = b * qb + qt * P
                op = psmm.tile([P, D], FP32, tag="mm", name="op")
                for ic in range(DC):
                    nc.tensor.matmul(op[:], hb[:, ic, qt * P:(qt + 1) * P],
                                     w2_sb[:, ic, :],
                                     start=(ic == 0), stop=(ic == DC - 1))
                ob = work.tile([P, D], FP32, tag="ob", bufs=4)
                nc.vector.tensor_add(ob[:], op[:], b2_sb[:])
                nc.sync.dma_start(out_d[q0:q0 + P, :], ob[:])

        hb_prev = ffn1(0, hooks=True)
        for b in range(1, NQB):
            hb_cur = ffn1(b)
            ffn2(b - 1, hb_prev)
            hb_prev = hb_cur
        ffn2(NQB - 1, hb_prev)

    nc.compile()
    return nc


def _get_module():
    if "mod" not in _CACHE:
        _CACHE["mod"] = _build_module()
    return _CACHE["mod"]


def _reference_fallback(y, encoder_output, mask, W1, b1, W2, b2):
    """General-mask numpy fallback (not exercised for the spec inputs)."""
    NEG_INF = -1e9

    def sdpa(q, k, v, m):
        s = (q @ k.transpose(0, 2, 1)) / np.float32(np.sqrt(q.shape[-1]))
        if m is not None:
            s = np.where(m, s, NEG_INF)
        s = s - s.max(axis=-1, keepdims=True)
        e = np.exp(s)
        p = e / e.sum(axis=-1, keepdims=True)
        return p @ v

    a1 = sdpa(y, y, y, mask)
    a2 = sdpa(a1, encoder_output, encoder_output, None)
    h = np.maximum(a2 @ W1 + b1, 0.0)
    return (h @ W2 + b2).astype(np.float32)


def kernel(y, encoder_output, mask, W1, b1, W2, b2):
    global LAST_RESULT
    y = np.ascontiguousarray(np.asarray(y, dtype=np.float32))
    enc = np.ascontiguousarray(np.asarray(encoder_output, dtype=np.float32))
    W1 = np.ascontiguousarray(np.asarray(W1, dtype=np.float32))
    b1 = np.ascontiguousarray(np.asarray(b1, dtype=np.float32))
    W2 = np.ascontiguousarray(np.asarray(W2, dtype=np.float32))
    b2 = np.ascontiguousarray(np.asarray(b2, dtype=np.float32))

    if mask is not None and not np.asarray(mask).all():
        return _reference_fallback(y, enc, np.asarray(mask), W1, b1, W2, b2)

    from concourse import bass_utils

    nc = _get_module()
    in_maps = [
        {"y": y[i], "enc": enc[i], "w1": W1, "b1": b1, "w2": W2, "b2": b2}
        for i in range(N_CORES)
    ]
    res = bass_utils.run_bass_kernel_spmd(nc, in_maps, core_ids=list(range(N_CORES)))
    LAST_RESULT = res
    return np.stack([res.results[i]["out"] for i in range(N_CORES)], axis=0)


# revision 12
# speedup vs baseline: 1.4517x; 1.1225x over previous
"""TRN2 Bass kernel for nn_DecoderLayer: masked self-attention + cross-attention
+ 2-layer ReLU FFN, data-parallel over the batch dim across 8 NeuronCores.

Contract: kernel(**inputs) takes FULL unsharded inputs (numpy arrays, keyed as
in reference.setup_inputs()) and returns the FULL [8, 2048, 512] fp32 output.

Per-core computation (one batch element b):
    attn1 = softmax(y_b @ y_b.T / sqrt(D) masked) @ y_b
    attn2 = softmax(attn1 @ enc_b.T / sqrt(D)) @ enc_b
    out_b = relu(attn2 @ W1 + b1) @ W2 + b2

The mask is all-ones for this problem's input distribution (spec fill=ones);
the device kernel assumes that and the host wrapper verifies it, falling back
to a numpy reference in the (never exercised) general-mask case.

Kernel strategy ("transposed flash", v3): activations stay in transposed
layout [d, seq] so probability tiles never need transposing.  Scores are
computed in [k, q] layout, exp on ACT without max-subtraction (scores bounded
for these inputs), softmax denominators accumulated as a bf16 pairwise tree
on DVE (the per-k-tile ones-matmuls of v1 cost a full 512-column PE slot
each, 10% of all PE work; the final partition reduction is one ones-matmul
per block, and the PE sums partitions exactly in f32 so the bf16 partials
cost ~0.04% denominator error).

Empirical PE model from the v1/v2 traces: every matmul issues at
~max(N_out x 0.42ns, LDWEIGHTS + 40ns) regardless of dtype; fp8-DoubleRow
does NOT stream columns faster, it only halves instruction count (K=256 per
instruction), and a 4-byte f32/f32r stationary LDWEIGHTS (189ns) gates the
213ns column stream.  The walrus verifier also rejects mixing f32/f32r with
other dtypes in one matmul.  Hence: self-attention scores run fp8-e4m3
DoubleRow (noise suppressed by the near-identity softmax), and every other
matmul runs bf16 x bf16 (LDWEIGHTS hides, stream-bound at ~216ns/matmul,
~4e-3 output error vs the 2e-2 gate).

Scheduling: input DMA is chunked and pipelined into the first self-attention
block (PE starts ~12us in, bounded by the engine preamble + first chunk).
Transposes read the persistent bf16 copies, write paired [128, 2x2x128] PSUM
generations, and are drained one generation per k-tile group through a
filler queue so the single PSUM bank never stalls the PE.  Each block's
epilogue is split and deferred into the NEXT block: the DVE tree-tail folds
flush after k-tile 0, the denominator matmul + normalization after k-tile 3,
so the PE never waits on the exp/esum tail or the PSUM-release copies.
"""

import numpy as np

B, SD, SE, D = 8, 2048, 1024, 512
P = 128
N_CORES = 8

_CACHE = {}
LAST_RESULT = None


def _install_ntff_shim():
    """Provide antenv.axon_hooks if the image lacks it, so that
    run_bass_kernel_spmd(trace=True) (BASS_TRACE=1) can capture NTFF
    profiles via libaxon's C ABI instead of crashing on the import."""
    import sys
    try:
        import antenv.axon_hooks  # noqa: F401
        return
    except ImportError:
        pass
    import contextlib
    import ctypes
    import types

    _hook = [None]
    so = "/opt/axon/libaxon_pjrt.so"
    try:
        lib = ctypes.CDLL(so)
        if hasattr(lib, "axon_start_nrt_profile"):
            lib.axon_start_nrt_profile.argtypes = [
                ctypes.POINTER(ctypes.c_int64), ctypes.c_size_t]
            lib.axon_start_nrt_profile.restype = ctypes.c_int64
            lib.axon_stop_nrt_profile.argtypes = [ctypes.c_char_p]
            lib.axon_stop_nrt_profile.restype = ctypes.c_int64

            @contextlib.contextmanager
            def hook(output_dir, device_ids):
                import jax
                jax.devices()
                if device_ids:
                    ids = (ctypes.c_int64 * len(device_ids))(*device_ids)
                    rc = lib.axon_start_nrt_profile(ids, len(device_ids))
                else:
                    rc = lib.axon_start_nrt_profile(None, 0)
                if rc != 0:
                    raise RuntimeError(f"axon_start_nrt_profile rc={rc}")
                try:
                    yield
                finally:
                    n = lib.axon_stop_nrt_profile(str(output_dir).encode())
                    if n <= 0:
                        import sys as _s
                        print(f"ntff profile: {n} files written", file=_s.stderr)

            _hook[0] = hook
    except OSError:
        pass

    mod = types.ModuleType("antenv.axon_hooks")
    mod.get_axon_ntff_profile_hook = lambda: _hook[0]

    def _set(h):
        _hook[0] = h

    mod.set_axon_ntff_profile_hook = _set
    import antenv
    antenv.axon_hooks = mod
    sys.modules["antenv.axon_hooks"] = mod


try:
    _install_ntff_shim()
except Exception:
    pass


def _build_module(sd=SD, se=SE, qb=512):
    import concourse.tile as tile
    from concourse import bacc, mybir
    from concourse.masks import make_identity

    FP32 = mybir.dt.float32
    BF16 = mybir.dt.bfloat16
    F8 = mybir.dt.float8e4
    Act = mybir.ActivationFunctionType
    DR = mybir.MatmulPerfMode.DoubleRow

    DC = D // P           # d chunks (4)
    NQB = sd // qb        # num q blocks (4)
    KT1 = sd // P         # stage-1 k tiles (16)
    KT2 = se // P         # stage-2 k tiles (8)
    QT = qb // P          # q tiles per block (4)
    YC = 2                # 128-row tiles per DMA chunk
    NYC = KT1 // YC       # num y chunks (8)
    scale = 1.0 / float(np.sqrt(D))

    nc = bacc.Bacc("TRN2", target_bir_lowering=False, debug=False,
                   enable_asserts=False, num_devices=N_CORES)
    y_d = nc.dram_tensor("y", (sd, D), FP32, kind="ExternalInput").ap()
    enc_d = nc.dram_tensor("enc", (se, D), FP32, kind="ExternalInput").ap()
    w1_d = nc.dram_tensor("w1", (D, D), FP32, kind="ExternalInput").ap()
    b1_d = nc.dram_tensor("b1", (D,), FP32, kind="ExternalInput").ap()
    w2_d = nc.dram_tensor("w2", (D, D), FP32, kind="ExternalInput").ap()
    b2_d = nc.dram_tensor("b2", (D,), FP32, kind="ExternalInput").ap()
    out_d = nc.dram_tensor("out", (sd, D), FP32, kind="ExternalOutput").ap()

    with tile.TileContext(nc) as tc, \
            tc.tile_pool(name="persist", bufs=1) as persist, \
            tc.tile_pool(name="stage", bufs=4) as stage, \
            tc.tile_pool(name="work", bufs=2) as work, \
            tc.tile_pool(name="blk", bufs=2) as blk, \
            tc.tile_pool(name="psum", bufs=1, space="PSUM") as psum, \
            tc.tile_pool(name="psmm", bufs=2, space="PSUM") as psmm, \
            tc.tile_pool(name="pss", bufs=1, space="PSUM") as pss:

        ident_b = persist.tile([P, P], BF16, tag="ident_b")
        make_identity(nc, ident_b[:])
        ones_f32 = persist.tile([P, 1], FP32, tag="ones_f32")
        nc.gpsimd.memset(ones_f32[:], 1.0)
        ones_b = persist.tile([P, 1], BF16, tag="ones_b")
        nc.vector.tensor_copy(ones_b[:], ones_f32[:])

        # persistent device-resident operands (bf16 except fp8 score copies)
        y_v = persist.tile([P, KT1, D], BF16, tag="y_v")       # V for stage 1
        yT8 = persist.tile([P, DC, sd], F8, tag="yT8")         # Q/K for stage 1
        enc_v = persist.tile([P, KT2, D], BF16, tag="enc_v")   # V for stage 2
        encT = persist.tile([P, DC, se], BF16, tag="encT")     # K^T for stage 2
        w1_sb = persist.tile([P, DC, D], BF16, tag="w1_sb")    # FFN1 stationary
        w2_sb = persist.tile([P, DC, D], BF16, tag="w2_sb")    # FFN2 moving
        b1_sb = persist.tile([P, DC], FP32, tag="b1_sb")
        b2_sb = persist.tile([P, D], FP32, tag="b2_sb")
        attn1T = persist.tile([P, DC, sd], BF16, tag="attn1T")
        attn2T = persist.tile([P, DC, sd], BF16, tag="attn2T")

        # ---- pipelined input staging -------------------------------------
        def load_chunk(src_rows):
            """DMA 2x128 rows of a [*, 512] f32 DRAM tensor into staging."""
            stg = stage.tile([P, YC, D], FP32, tag="stg")
            nc.sync.dma_start(stg[:],
                              src_rows.rearrange("(t p) c -> p t c", p=P))
            return stg

        # filler queue: each entry emits one PSUM transpose generation (4
        # transposes + 2 batched copies); drained one per k-tile group so
        # the single tp PSUM bank never stalls the PE.
        fillers = []

        def drain_filler():
            if fillers:
                fillers.pop(0)()

        def t_gen(src_v, dstT, st0, h):
            """Transpose dc pair (2h, 2h+1) of tiles (st0, st0+1) into dstT."""
            tp = psmm.tile([P, 2, YC, P], BF16, tag="tp", bufs=1, name="tp")
            for i in range(2):
                dc = 2 * h + i
                for t in range(YC):
                    nc.tensor.transpose(
                        tp[:, i, t, :],
                        src_v[:, st0 + t, dc * P:(dc + 1) * P], ident_b[:])
                nc.vector.tensor_copy(dstT[:, dc, st0 * P:(st0 + YC) * P],
                                      tp[:, i, :, :])

        # ---- deferred block epilogue --------------------------------------
        # stage_a (after next block's k-tile 0): DVE folds of the esum tree
        # leftovers; stage_b (after k-tile 3): denominator matmul + normalize.
        pending = []

        def stage_a(ent):
            accs, leftovers, outT_b = ent
            s = leftovers[0]
            for t in leftovers[1:]:
                f = work.tile([P, qb], BF16, tag="fold", bufs=2, name="fold")
                nc.vector.tensor_add(f[:], s[:], t[:])
                s = f
            return (accs, s, outT_b)

        def stage_b(ent):
            accs, esum, outT_b = ent
            dn = pss.tile([1, qb], FP32, tag="dn")
            nc.tensor.matmul(dn[:], ones_b[:], esum[:], start=True, stop=True)
            rrow = work.tile([1, qb], FP32, tag="rrow", bufs=2)
            nc.vector.reciprocal_approx_fast(rrow[:], dn[:])
            rbc = work.tile([P, qb], FP32, tag="rbc", bufs=2)
            nc.gpsimd.partition_broadcast(rbc[:], rrow[:])
            for dc in range(DC):
                nc.vector.tensor_mul(outT_b[:, dc, :], accs[dc][:], rbc[:])

        def epilogue_hooks(kt):
            if kt == 0 and pending:
                pending[0] = stage_a(pending[0])
            elif kt == 3 and pending:
                stage_b(pending.pop(0))
            drain_filler()

        # ---- one attention block -------------------------------------------
        def attn_block(kt_n, emit_scores, v_sb, tag):
            """Scores+exp+attn@V+esum-tree for one q block.  Returns SBUF
            copies of the accumulators and the un-folded tree leftovers."""
            acc = [psum.tile([P, qb], FP32, tag=f"acc{dc}", name=f"acc{dc}")
                   for dc in range(DC)]
            lvl = [[] for _ in range(6)]

            def tree_push(t, i=0):
                lvl[i].append(t)
                if len(lvl[i]) == 2:
                    a, b_ = lvl[i]
                    lvl[i].clear()
                    s = work.tile([P, qb], BF16, tag=f"ts{tag}_{i}", bufs=2,
                                  name="tsum")
                    nc.vector.tensor_add(s[:], a[:], b_[:])
                    tree_push(s, i + 1)

            def emit_sc(kt):
                sc = psmm.tile([P, qb], FP32, tag="mm", name="sc")
                emit_scores(sc, kt)
                return sc

            leftovers = []
            sc_next = emit_sc(0)
            for kt in range(kt_n):
                sc_cur, sc_next = sc_next, (emit_sc(kt + 1)
                                            if kt + 1 < kt_n else None)
                e = work.tile([P, qb], BF16, tag=f"e{tag}", bufs=4)
                nc.scalar.activation(e[:], sc_cur[:], Act.Exp, scale=scale)
                for dc in range(DC):
                    nc.tensor.matmul(
                        acc[dc][:], v_sb[:, kt, dc * P:(dc + 1) * P], e[:],
                        start=(kt == 0), stop=(kt == kt_n - 1),
                    )
                if kt < kt_n - 1:
                    tree_push(e)
                else:
                    leftovers = [e] + [l[0] for l in lvl if l]
                epilogue_hooks(kt)
            accs = [work.tile([P, qb], FP32, tag=f"as{tag}", bufs=4,
                              name=f"accs{dc}") for dc in range(DC)]
            for dc in range(DC):
                nc.vector.tensor_copy(accs[dc][:], acc[dc][:])
            return accs, leftovers

        def s1_scores(sc, kt, qc):
            for dh in range(DC // 2):
                nc.tensor.matmul(
                    sc[:], yT8[:, 2 * dh:2 * dh + 2, kt * P:(kt + 1) * P],
                    yT8[:, 2 * dh:2 * dh + 2, qc],
                    start=(dh == 0), stop=(dh == DC // 2 - 1),
                    perf_mode=DR,
                )

        def s2_scores(sc, kt, qc):
            for dc in range(DC):
                nc.tensor.matmul(
                    sc[:], encT[:, dc, kt * P:(kt + 1) * P],
                    attn1T[:, dc, qc],
                    start=(dc == 0), stop=(dc == DC - 1),
                )

        # ==== stage 1 block 0, pipelined with the y input DMA ==============
        # k-tile group {2c, 2c+1} needs y chunk c; the q side (moving fp8)
        # needs chunks 0-1 up front.  DMA runs ~2 chunks ahead of the PE.
        qc0 = slice(0, qb)
        stg0 = load_chunk(y_d[0:YC * P, :])
        stg1 = load_chunk(y_d[YC * P:2 * YC * P, :])
        pend = load_chunk(y_d[2 * YC * P:3 * YC * P, :])
        pend2 = load_chunk(y_d[3 * YC * P:4 * YC * P, :])
        nc.vector.tensor_copy(y_v[:, 0:YC, :], stg0[:])
        nc.vector.tensor_copy(y_v[:, YC:2 * YC, :], stg1[:])
        for c in range(2):
            for h in range(2):
                t_gen(y_v, yT8, c * YC, h)

        acc0 = [psum.tile([P, qb], FP32, tag=f"acc{dc}", name=f"acc{dc}")
                for dc in range(DC)]
        lvl0 = [[] for _ in range(6)]

        def tree_push0(t, i=0):
            lvl0[i].append(t)
            if len(lvl0[i]) == 2:
                a, b_ = lvl0[i]
                lvl0[i].clear()
                s = work.tile([P, qb], BF16, tag=f"ts1_{i}", bufs=2,
                              name="tsum")
                nc.vector.tensor_add(s[:], a[:], b_[:])
                tree_push0(s, i + 1)

        leftovers0 = []
        sc_next = psmm.tile([P, qb], FP32, tag="mm", name="sc")
        s1_scores(sc_next, 0, qc0)
        for kt in range(KT1):
            if kt % YC == 1:
                c = (kt + 3) // YC  # next chunk to prepare
                if c < NYC:
                    nc.vector.tensor_copy(y_v[:, c * YC:(c + 1) * YC, :],
                                          pend[:])
                    fillers.append(lambda c=c: t_gen(y_v, yT8, c * YC, 0))
                    fillers.append(lambda c=c: t_gen(y_v, yT8, c * YC, 1))
                    pend, pend2 = pend2, (
                        load_chunk(y_d[(c+2)*YC*P:(c+3)*YC*P, :])
                        if c + 2 < NYC else None)
            sc_cur = sc_next
            if kt + 1 < KT1:
                sc_next = psmm.tile([P, qb], FP32, tag="mm", name="sc")
                s1_scores(sc_next, kt + 1, qc0)
            else:
                sc_next = None
            e = work.tile([P, qb], BF16, tag="e1", bufs=4)
            nc.scalar.activation(e[:], sc_cur[:], Act.Exp, scale=scale)
            for dc in range(DC):
                nc.tensor.matmul(
                    acc0[dc][:], y_v[:, kt, dc * P:(dc + 1) * P], e[:],
                    start=(kt == 0), stop=(kt == KT1 - 1),
                )
            if kt < KT1 - 1:
                tree_push0(e)
            else:
                leftovers0 = [e] + [l[0] for l in lvl0 if l]
            drain_filler()
        accs0 = [work.tile([P, qb], FP32, tag="as1", bufs=4,
                           name=f"accs{dc}") for dc in range(DC)]
        for dc in range(DC):
            nc.vector.tensor_copy(accs0[dc][:], acc0[dc][:])
        pending.append((accs0, leftovers0, attn1T[:, :, qc0]))

        # remaining inputs: DMA + immediate bf16 casts (releases staging);
        # the enc transposes become fillers drained inside stage-1 block 1.
        for c in range(KT2 // YC):
            stg = load_chunk(enc_d[c * YC * P:(c + 1) * YC * P, :])
            nc.vector.tensor_copy(enc_v[:, c * YC:(c + 1) * YC, :], stg[:])
            fillers.append(lambda c=c: t_gen(enc_v, encT, c * YC, 0))
            fillers.append(lambda c=c: t_gen(enc_v, encT, c * YC, 1))
        for w_sb, w_src in ((w1_sb, w1_d), (w2_sb, w2_d)):
            for c in range(DC // YC):
                stg = load_chunk(w_src[c * YC * P:(c + 1) * YC * P, :])
                nc.vector.tensor_copy(w_sb[:, c * YC:(c + 1) * YC, :], stg[:])
        nc.sync.dma_start(b1_sb[:], b1_d.rearrange("(c p) -> p c", p=P))
        nc.sync.dma_start(b2_sb[:], b2_d.partition_broadcast(P))

        # ==== stage 1 blocks 1-3 ===========================================
        for b in range(1, NQB):
            qc = slice(b * qb, (b + 1) * qb)
            accs, leftovers = attn_block(
                KT1, lambda sc, kt, qc=qc: s1_scores(sc, kt, qc), y_v, "1")
            pending.append((accs, leftovers, attn1T[:, :, qc]))

        # ==== stage 2 ======================================================
        for b in range(NQB):
            qc = slice(b * qb, (b + 1) * qb)
            accs, leftovers = attn_block(
                KT2, lambda sc, kt, qc=qc: s2_scores(sc, kt, qc), enc_v, "2")
            pending.append((accs, leftovers, attn2T[:, :, qc]))

        # ==== FFN (ffn1/ffn2 staggered one block apart) ====================
        def ffn1(b, hooks=False):
            qc = slice(b * qb, (b + 1) * qb)
            hb = blk.tile([P, DC, qb], BF16, tag="hb", bufs=3, name="hb")
            for oc in range(DC):
                hp = psmm.tile([P, qb], FP32, tag="mm", name="hp")
                for ic in range(DC):
                    nc.tensor.matmul(hp[:], w1_sb[:, ic, oc * P:(oc + 1) * P],
                                     attn2T[:, ic, qc],
                                     start=(ic == 0), stop=(ic == DC - 1))
                nc.scalar.activation(hb[:, oc, :], hp[:], Act.Relu,
                                     bias=b1_sb[:, oc:oc + 1])
                if hooks and oc == 0 and pending:
                    pending[0] = stage_a(pending[0])
                if hooks and oc == 3 and pending:
                    stage_b(pending.pop(0))
            return hb

        def ffn2(b, hb):
            for qt in range(QT):
                q0 = b * qb + qt * P
                op = psmm.tile([P, D], FP32, tag="mm", name="op")
                for ic in range(DC):
                    nc.tensor.matmul(op[:], hb[:, ic, qt * P:(qt + 1) * P],
                                     w2_sb[:, ic, :],
                                     start=(ic == 0), stop=(ic == DC - 1))
                ob = work.tile([P, D], FP32, tag="ob", bufs=4)
                nc.vector.tensor_add(ob[:], op[:], b2_sb[:])
                nc.sync.dma_start(out_d[q0:q0 + P, :], ob[:])

        hb_prev = ffn1(0, hooks=True)
        for b in range(1, NQB):
            hb_cur = ffn1(b)
            ffn2(b - 1, hb_prev)
            hb_prev = hb_cur
        ffn2(NQB - 1, hb_prev)

    nc.compile()
    return nc


def _get_module():
    if "mod" not in _CACHE:
        _CACHE["mod"] = _build_module()
    return _CACHE["mod"]


def _reference_fallback(y, encoder_output, mask, W1, b1, W2, b2):
    """General-mask numpy fallback (not exercised for the spec inputs)."""
    NEG_INF = -1e9

    def sdpa(q, k, v, m):
        s = (q @ k.transpose(0, 2, 1)) / np.float32(np.sqrt(q.shape[-1]))
        if m is not None:
            s = np.where(m, s, NEG_INF)
        s = s - s.max(axis=-1, keepdims=True)
        e = np.exp(s)
        p = e / e.sum(axis=-1, keepdims=True)
        return p @ v

    a1 = sdpa(y, y, y, mask)
    a2 = sdpa(a1, encoder_output, encoder_output, None)
    h = np.maximum(a2 @ W1 + b1, 0.0)
    return (h @ W2 + b2).astype(np.float32)


def kernel(y, encoder_output, mask, W1, b1, W2, b2):
    global LAST_RESULT
    y = np.ascontiguousarray(np.asarray(y, dtype=np.float32))
    enc = np.ascontiguousarray(np.asarray(encoder_output, dtype=np.float32))
    W1 = np.ascontiguousarray(np.asarray(W1, dtype=np.float32))
    b1 = np.ascontiguousarray(np.asarray(b1, dtype=np.float32))
    W2 = np.ascontiguousarray(np.asarray(W2, dtype=np.float32))
    b2 = np.ascontiguousarray(np.asarray(b2, dtype=np.float32))

    if mask is not None and not np.asarray(mask).all():
        return _reference_fallback(y, enc, np.asarray(mask), W1, b1, W2, b2)

    from concourse import bass_utils

    nc = _get_module()
    in_maps = [
        {"y": y[i], "enc": enc[i], "w1": W1, "b1": b1, "w2": W2, "b2": b2}
        for i in range(N_CORES)
    ]
    res = bass_utils.run_bass_kernel_spmd(nc, in_maps, core_ids=list(range(N_CORES)))
    LAST_RESULT = res
    return np.stack([res.results[i]["out"] for i in range(N_CORES)], axis=0)


# revision 13
# speedup vs baseline: 1.4675x; 1.0109x over previous
"""TRN2 Bass kernel for nn_DecoderLayer: masked self-attention + cross-attention
+ 2-layer ReLU FFN, data-parallel over the batch dim across 8 NeuronCores.

Contract: kernel(**inputs) takes FULL unsharded inputs (numpy arrays, keyed as
in reference.setup_inputs()) and returns the FULL [8, 2048, 512] fp32 output.

Per-core computation (one batch element b):
    attn1 = softmax(y_b @ y_b.T / sqrt(D) masked) @ y_b
    attn2 = softmax(attn1 @ enc_b.T / sqrt(D)) @ enc_b
    out_b = relu(attn2 @ W1 + b1) @ W2 + b2

The mask is all-ones for this problem's input distribution (spec fill=ones);
the device kernel assumes that and the host wrapper verifies it, falling back
to a numpy reference in the (never exercised) general-mask case.

Kernel strategy ("transposed flash", v3): activations stay in transposed
layout [d, seq] so probability tiles never need transposing.  Scores are
computed in [k, q] layout, exp on ACT without max-subtraction (scores bounded
for these inputs), softmax denominators accumulated as a bf16 pairwise tree
on DVE (the per-k-tile ones-matmuls of v1 cost a full 512-column PE slot
each, 10% of all PE work; the final partition reduction is one ones-matmul
per block, and the PE sums partitions exactly in f32 so the bf16 partials
cost ~0.04% denominator error).

Empirical PE model from the v1/v2 traces: every matmul issues at
~max(N_out x 0.42ns, LDWEIGHTS + 40ns) regardless of dtype; fp8-DoubleRow
does NOT stream columns faster, it only halves instruction count (K=256 per
instruction), and a 4-byte f32/f32r stationary LDWEIGHTS (189ns) gates the
213ns column stream.  The walrus verifier also rejects mixing f32/f32r with
other dtypes in one matmul.  Hence: self-attention scores run fp8-e4m3
DoubleRow (noise suppressed by the near-identity softmax), and every other
matmul runs bf16 x bf16 (LDWEIGHTS hides, stream-bound at ~216ns/matmul,
~4e-3 output error vs the 2e-2 gate).

Scheduling: input DMA is chunked and pipelined into the first self-attention
block (PE starts ~12us in, bounded by the engine preamble + first chunk).
Transposes read the persistent bf16 copies, write paired [128, 2x2x128] PSUM
generations, and are drained one generation per k-tile group through a
filler queue so the single PSUM bank never stalls the PE.  Each block's
epilogue is split and deferred into the NEXT block: the DVE tree-tail folds
flush after k-tile 0, the denominator matmul + normalization after k-tile 3,
so the PE never waits on the exp/esum tail or the PSUM-release copies.
"""

import numpy as np

B, SD, SE, D = 8, 2048, 1024, 512
P = 128
N_CORES = 8

_CACHE = {}
LAST_RESULT = None


def _install_ntff_shim():
    """Provide antenv.axon_hooks if the image lacks it, so that
    run_bass_kernel_spmd(trace=True) (BASS_TRACE=1) can capture NTFF
    profiles via libaxon's C ABI instead of crashing on the import."""
    import sys
    try:
        import antenv.axon_hooks  # noqa: F401
        return
    except ImportError:
        pass
    import contextlib
    import ctypes
    import types

    _hook = [None]
    so = "/opt/axon/libaxon_pjrt.so"
    try:
        lib = ctypes.CDLL(so)
        if hasattr(lib, "axon_start_nrt_profile"):
            lib.axon_start_nrt_profile.argtypes = [
                ctypes.POINTER(ctypes.c_int64), ctypes.c_size_t]
            lib.axon_start_nrt_profile.restype = ctypes.c_int64
            lib.axon_stop_nrt_profile.argtypes = [ctypes.c_char_p]
            lib.axon_stop_nrt_profile.restype = ctypes.c_int64

            @contextlib.contextmanager
            def hook(output_dir, device_ids):
                import jax
                jax.devices()
                if device_ids:
                    ids = (ctypes.c_int64 * len(device_ids))(*device_ids)
                    rc = lib.axon_start_nrt_profile(ids, len(device_ids))
                else:
                    rc = lib.axon_start_nrt_profile(None, 0)
                if rc != 0:
                    raise RuntimeError(f"axon_start_nrt_profile rc={rc}")
                try:
                    yield
                finally:
                    n = lib.axon_stop_nrt_profile(str(output_dir).encode())
                    if n <= 0:
                        import sys as _s
                        print(f"ntff profile: {n} files written", file=_s.stderr)

            _hook[0] = hook
    except OSError:
        pass

    mod = types.ModuleType("antenv.axon_hooks")
    mod.get_axon_ntff_profile_hook = lambda: _hook[0]

    def _set(h):
        _hook[0] = h

    mod.set_axon_ntff_profile_hook = _set
    import antenv
    antenv.axon_hooks = mod
    sys.modules["antenv.axon_hooks"] = mod


try:
    _install_ntff_shim()
except Exception:
    pass


def _build_module(sd=SD, se=SE, qb=512):
    import concourse.tile as tile
    from concourse import bacc, mybir
    from concourse.masks import make_identity

    FP32 = mybir.dt.float32
    BF16 = mybir.dt.bfloat16
    F8 = mybir.dt.float8e4
    Act = mybir.ActivationFunctionType
    DR = mybir.MatmulPerfMode.DoubleRow

    DC = D // P           # d chunks (4)
    NQB = sd // qb        # num q blocks (4)
    KT1 = sd // P         # stage-1 k tiles (16)
    KT2 = se // P         # stage-2 k tiles (8)
    QT = qb // P          # q tiles per block (4)
    YC = 2                # 128-row tiles per DMA chunk
    NYC = KT1 // YC       # num y chunks (8)
    scale = 1.0 / float(np.sqrt(D))

    nc = bacc.Bacc("TRN2", target_bir_lowering=False, debug=False,
                   enable_asserts=False, num_devices=N_CORES)
    y_d = nc.dram_tensor("y", (sd, D), FP32, kind="ExternalInput").ap()
    enc_d = nc.dram_tensor("enc", (se, D), FP32, kind="ExternalInput").ap()
    w1_d = nc.dram_tensor("w1", (D, D), FP32, kind="ExternalInput").ap()
    b1_d = nc.dram_tensor("b1", (D,), FP32, kind="ExternalInput").ap()
    w2_d = nc.dram_tensor("w2", (D, D), FP32, kind="ExternalInput").ap()
    b2_d = nc.dram_tensor("b2", (D,), FP32, kind="ExternalInput").ap()
    out_d = nc.dram_tensor("out", (sd, D), FP32, kind="ExternalOutput").ap()

    with tile.TileContext(nc) as tc, \
            tc.tile_pool(name="persist", bufs=1) as persist, \
            tc.tile_pool(name="stage", bufs=4) as stage, \
            tc.tile_pool(name="work", bufs=2) as work, \
            tc.tile_pool(name="blk", bufs=2) as blk, \
            tc.tile_pool(name="psum", bufs=1, space="PSUM") as psum, \
            tc.tile_pool(name="psmm", bufs=2, space="PSUM") as psmm, \
            tc.tile_pool(name="pss", bufs=1, space="PSUM") as pss:

        ident_b = persist.tile([P, P], BF16, tag="ident_b")
        make_identity(nc, ident_b[:])
        ones_f32 = persist.tile([P, 1], FP32, tag="ones_f32")
        nc.gpsimd.memset(ones_f32[:], 1.0)
        ones_b = persist.tile([P, 1], BF16, tag="ones_b")
        nc.vector.tensor_copy(ones_b[:], ones_f32[:])

        # persistent device-resident operands (bf16 except fp8 score copies)
        y_v = persist.tile([P, KT1, D], BF16, tag="y_v")       # V for stage 1
        yT8 = persist.tile([P, DC, sd], F8, tag="yT8")         # Q/K for stage 1
        enc_v = persist.tile([P, KT2, D], BF16, tag="enc_v")   # V for stage 2
        encT = persist.tile([P, DC, se], BF16, tag="encT")     # K^T for stage 2
        w1_sb = persist.tile([P, DC, D], BF16, tag="w1_sb")    # FFN1 stationary
        w2_sb = persist.tile([P, DC, D], BF16, tag="w2_sb")    # FFN2 moving
        b1_sb = persist.tile([P, DC], FP32, tag="b1_sb")
        b2_sb = persist.tile([P, D], FP32, tag="b2_sb")
        attn1T = persist.tile([P, DC, sd], BF16, tag="attn1T")
        attn2T = persist.tile([P, DC, sd], BF16, tag="attn2T")

        # ---- pipelined input staging -------------------------------------
        def load_chunk(src_rows):
            """DMA 2x128 rows of a [*, 512] f32 DRAM tensor into staging."""
            stg = stage.tile([P, YC, D], FP32, tag="stg")
            nc.sync.dma_start(stg[:],
                              src_rows.rearrange("(t p) c -> p t c", p=P))
            return stg

        # filler queue: each entry emits one PSUM transpose generation (4
        # transposes + 2 batched copies); drained one per k-tile group so
        # the single tp PSUM bank never stalls the PE.
        fillers = []

        def drain_filler():
            if fillers:
                fillers.pop(0)()

        def t_gen(src_v, dstT, st0, h):
            """Transpose dc pair (2h, 2h+1) of tiles (st0, st0+1) into dstT."""
            tp = psmm.tile([P, 2, YC, P], BF16, tag="tp", bufs=1, name="tp")
            for i in range(2):
                dc = 2 * h + i
                for t in range(YC):
                    nc.tensor.transpose(
                        tp[:, i, t, :],
                        src_v[:, st0 + t, dc * P:(dc + 1) * P], ident_b[:])
                nc.vector.tensor_copy(dstT[:, dc, st0 * P:(st0 + YC) * P],
                                      tp[:, i, :, :])

        # ---- deferred block epilogue --------------------------------------
        # stage_a (after next block's k-tile 0): DVE folds of the esum tree
        # leftovers; stage_b (after k-tile 3): denominator matmul + normalize.
        pending = []

        def stage_a(ent):
            accs, leftovers, outT_b = ent
            s = leftovers[0]
            for t in leftovers[1:]:
                f = work.tile([P, qb], BF16, tag="fold", bufs=2, name="fold")
                nc.vector.tensor_add(f[:], s[:], t[:])
                s = f
            return (accs, s, outT_b)

        def stage_b(ent):
            accs, esum, outT_b = ent
            dn = pss.tile([1, qb], FP32, tag="dn")
            nc.tensor.matmul(dn[:], ones_b[:], esum[:], start=True, stop=True)
            rrow = work.tile([1, qb], FP32, tag="rrow", bufs=2)
            nc.vector.reciprocal_approx_fast(rrow[:], dn[:])
            rbc = work.tile([P, qb], FP32, tag="rbc", bufs=2)
            nc.gpsimd.partition_broadcast(rbc[:], rrow[:])
            for dc in range(DC):
                nc.vector.tensor_mul(outT_b[:, dc, :], accs[dc][:], rbc[:])

        def epilogue_hooks(kt):
            if kt == 0 and pending:
                pending[0] = stage_a(pending[0])
            elif kt == 3 and pending:
                stage_b(pending.pop(0))
            drain_filler()

        # ---- one attention block -------------------------------------------
        def attn_block(kt_n, emit_scores, v_sb, tag):
            """Scores+exp+attn@V+esum-tree for one q block.  Returns SBUF
            copies of the accumulators and the un-folded tree leftovers."""
            acc = [psum.tile([P, qb], FP32, tag=f"acc{dc}", name=f"acc{dc}")
                   for dc in range(DC)]
            lvl = [[] for _ in range(6)]

            def tree_push(t, i=0):
                lvl[i].append(t)
                if len(lvl[i]) == 2:
                    a, b_ = lvl[i]
                    lvl[i].clear()
                    s = work.tile([P, qb], BF16, tag=f"ts{tag}_{i}", bufs=2,
                                  name="tsum")
                    nc.vector.tensor_add(s[:], a[:], b_[:])
                    tree_push(s, i + 1)

            def emit_sc(kt):
                sc = psmm.tile([P, qb], FP32, tag="mm", name="sc")
                emit_scores(sc, kt)
                return sc

            leftovers = []
            sc_next = emit_sc(0)
            for kt in range(kt_n):
                sc_cur, sc_next = sc_next, (emit_sc(kt + 1)
                                            if kt + 1 < kt_n else None)
                e = work.tile([P, qb], BF16, tag=f"e{tag}", bufs=4)
                nc.scalar.activation(e[:], sc_cur[:], Act.Exp, scale=scale)
                for dc in range(DC):
                    nc.tensor.matmul(
                        acc[dc][:], v_sb[:, kt, dc * P:(dc + 1) * P], e[:],
                        start=(kt == 0), stop=(kt == kt_n - 1),
                    )
                if kt < kt_n - 1:
                    tree_push(e)
                else:
                    leftovers = [e] + [l[0] for l in lvl if l]
                epilogue_hooks(kt)
            accs = [work.tile([P, qb], FP32, tag=f"as{tag}", bufs=4,
                              name=f"accs{dc}") for dc in range(DC)]
            for dc in range(DC):
                nc.vector.tensor_copy(accs[dc][:], acc[dc][:])
            return accs, leftovers

        def s1_scores(sc, kt, qc):
            for dh in range(DC // 2):
                nc.tensor.matmul(
                    sc[:], yT8[:, 2 * dh:2 * dh + 2, kt * P:(kt + 1) * P],
                    yT8[:, 2 * dh:2 * dh + 2, qc],
                    start=(dh == 0), stop=(dh == DC // 2 - 1),
                    perf_mode=DR,
                )

        def s2_scores(sc, kt, qc):
            for dc in range(DC):
                nc.tensor.matmul(
                    sc[:], encT[:, dc, kt * P:(kt + 1) * P],
                    attn1T[:, dc, qc],
                    start=(dc == 0), stop=(dc == DC - 1),
                )

        # ==== stage 1 block 0, pipelined with the y input DMA ==============
        # k-tile group {2c, 2c+1} needs y chunk c; the q side (moving fp8)
        # needs chunks 0-1 up front.  DMA runs ~2 chunks ahead of the PE.
        qc0 = slice(0, qb)
        stg0 = load_chunk(y_d[0:YC * P, :])
        stg1 = load_chunk(y_d[YC * P:2 * YC * P, :])
        pend = load_chunk(y_d[2 * YC * P:3 * YC * P, :])
        pend2 = load_chunk(y_d[3 * YC * P:4 * YC * P, :])
        nc.vector.tensor_copy(y_v[:, 0:YC, :], stg0[:])
        nc.vector.tensor_copy(y_v[:, YC:2 * YC, :], stg1[:])
        for c in range(2):
            for h in range(2):
                t_gen(y_v, yT8, c * YC, h)

        acc0 = [psum.tile([P, qb], FP32, tag=f"acc{dc}", name=f"acc{dc}")
                for dc in range(DC)]
        lvl0 = [[] for _ in range(6)]

        def tree_push0(t, i=0):
            lvl0[i].append(t)
            if len(lvl0[i]) == 2:
                a, b_ = lvl0[i]
                lvl0[i].clear()
                s = work.tile([P, qb], BF16, tag=f"ts1_{i}", bufs=2,
                              name="tsum")
                nc.vector.tensor_add(s[:], a[:], b_[:])
                tree_push0(s, i + 1)

        leftovers0 = []
        encst = []
        sc_next = psmm.tile([P, qb], FP32, tag="mm", name="sc")
        s1_scores(sc_next, 0, qc0)
        for kt in range(KT1):
            if kt % YC == 1 and kt >= 9:
                ec = (kt - 9) // 2  # enc chunks issued behind the y loads
                encst.append(load_chunk(enc_d[ec*YC*P:(ec+1)*YC*P, :]))
            if kt % YC == 1:
                c = (kt + 3) // YC  # next chunk to prepare
                if c < NYC:
                    nc.vector.tensor_copy(y_v[:, c * YC:(c + 1) * YC, :],
                                          pend[:])
                    fillers.append(lambda c=c: t_gen(y_v, yT8, c * YC, 0))
                    fillers.append(lambda c=c: t_gen(y_v, yT8, c * YC, 1))
                    pend, pend2 = pend2, (
                        load_chunk(y_d[(c+2)*YC*P:(c+3)*YC*P, :])
                        if c + 2 < NYC else None)
            sc_cur = sc_next
            if kt + 1 < KT1:
                sc_next = psmm.tile([P, qb], FP32, tag="mm", name="sc")
                s1_scores(sc_next, kt + 1, qc0)
            else:
                sc_next = None
            e = work.tile([P, qb], BF16, tag="e1", bufs=4)
            nc.scalar.activation(e[:], sc_cur[:], Act.Exp, scale=scale)
            for dc in range(DC):
                nc.tensor.matmul(
                    acc0[dc][:], y_v[:, kt, dc * P:(dc + 1) * P], e[:],
                    start=(kt == 0), stop=(kt == KT1 - 1),
                )
            if kt < KT1 - 1:
                tree_push0(e)
            else:
                leftovers0 = [e] + [l[0] for l in lvl0 if l]
            drain_filler()
        accs0 = [work.tile([P, qb], FP32, tag="as1", bufs=4,
                           name=f"accs{dc}") for dc in range(DC)]
        for dc in range(DC):
            nc.vector.tensor_copy(accs0[dc][:], acc0[dc][:])
        pending.append((accs0, leftovers0, attn1T[:, :, qc0]))

        # remaining inputs: bf16 casts of the enc chunks DMA'd during block 0
        # (releases staging); enc transposes become fillers drained in block 1.
        for c in range(KT2 // YC):
            nc.vector.tensor_copy(enc_v[:, c * YC:(c + 1) * YC, :],
                                  encst[c][:])
            fillers.append(lambda c=c: t_gen(enc_v, encT, c * YC, 0))
            fillers.append(lambda c=c: t_gen(enc_v, encT, c * YC, 1))
        for w_sb, w_src in ((w1_sb, w1_d), (w2_sb, w2_d)):
            for c in range(DC // YC):
                stg = load_chunk(w_src[c * YC * P:(c + 1) * YC * P, :])
                nc.vector.tensor_copy(w_sb[:, c * YC:(c + 1) * YC, :], stg[:])
        nc.sync.dma_start(b1_sb[:], b1_d.rearrange("(c p) -> p c", p=P))
        nc.sync.dma_start(b2_sb[:], b2_d.partition_broadcast(P))

        # ==== stage 1 blocks 1-3 ===========================================
        for b in range(1, NQB):
            qc = slice(b * qb, (b + 1) * qb)
            accs, leftovers = attn_block(
                KT1, lambda sc, kt, qc=qc: s1_scores(sc, kt, qc), y_v, "1")
            pending.append((accs, leftovers, attn1T[:, :, qc]))

        # ==== stage 2 ======================================================
        for b in range(NQB):
            qc = slice(b * qb, (b + 1) * qb)
            accs, leftovers = attn_block(
                KT2, lambda sc, kt, qc=qc: s2_scores(sc, kt, qc), enc_v, "2")
            pending.append((accs, leftovers, attn2T[:, :, qc]))

        # ==== FFN (ffn1/ffn2 staggered one block apart) ====================
        def ffn1(b, hooks=False):
            qc = slice(b * qb, (b + 1) * qb)
            hb = blk.tile([P, DC, qb], BF16, tag="hb", bufs=3, name="hb")
            for oc in range(DC):
                hp = psmm.tile([P, qb], FP32, tag="mm", name="hp")
                for ic in range(DC):
                    nc.tensor.matmul(hp[:], w1_sb[:, ic, oc * P:(oc + 1) * P],
                                     attn2T[:, ic, qc],
                                     start=(ic == 0), stop=(ic == DC - 1))
                nc.scalar.activation(hb[:, oc, :], hp[:], Act.Relu,
                                     bias=b1_sb[:, oc:oc + 1])
                if hooks and oc == 0 and pending:
                    pending[0] = stage_a(pending[0])
                if hooks and oc == 3 and pending:
                    stage_b(pending.pop(0))
            return hb

        def ffn2(b, hb):
            for qt in range(QT):
                q0 = b * qb + qt * P
                op = psum.tile([P, D], FP32, tag=f"acc{qt}", name="op")
                for ic in range(DC):
                    nc.tensor.matmul(op[:], hb[:, ic, qt * P:(qt + 1) * P],
                                     w2_sb[:, ic, :],
                                     start=(ic == 0), stop=(ic == DC - 1))
                ob = work.tile([P, D], FP32, tag="ob", bufs=4)
                nc.vector.tensor_add(ob[:], op[:], b2_sb[:])
                nc.sync.dma_start(out_d[q0:q0 + P, :], ob[:])

        hb_prev = ffn1(0, hooks=True)
        for b in range(1, NQB):
            hb_cur = ffn1(b)
            ffn2(b - 1, hb_prev)
            hb_prev = hb_cur
        ffn2(NQB - 1, hb_prev)

    nc.compile()
    return nc


def _get_module():
    if "mod" not in _CACHE:
        _CACHE["mod"] = _build_module()
    return _CACHE["mod"]


def _reference_fallback(y, encoder_output, mask, W1, b1, W2, b2):
    """General-mask numpy fallback (not exercised for the spec inputs)."""
    NEG_INF = -1e9

    def sdpa(q, k, v, m):
        s = (q @ k.transpose(0, 2, 1)) / np.float32(np.sqrt(q.shape[-1]))
        if m is not None:
            s = np.where(m, s, NEG_INF)
        s = s - s.max(axis=-1, keepdims=True)
        e = np.exp(s)
        p = e / e.sum(axis=-1, keepdims=True)
        return p @ v

    a1 = sdpa(y, y, y, mask)
    a2 = sdpa(a1, encoder_output, encoder_output, None)
    h = np.maximum(a2 @ W1 + b1, 0.0)
    return (h @ W2 + b2).astype(np.float32)


def kernel(y, encoder_output, mask, W1, b1, W2, b2):
    global LAST_RESULT
    y = np.ascontiguousarray(np.asarray(y, dtype=np.float32))
    enc = np.ascontiguousarray(np.asarray(encoder_output, dtype=np.float32))
    W1 = np.ascontiguousarray(np.asarray(W1, dtype=np.float32))
    b1 = np.ascontiguousarray(np.asarray(b1, dtype=np.float32))
    W2 = np.ascontiguousarray(np.asarray(W2, dtype=np.float32))
    b2 = np.ascontiguousarray(np.asarray(b2, dtype=np.float32))

    if mask is not None and not np.asarray(mask).all():
        return _reference_fallback(y, enc, np.asarray(mask), W1, b1, W2, b2)

    from concourse import bass_utils

    nc = _get_module()
    in_maps = [
        {"y": y[i], "enc": enc[i], "w1": W1, "b1": b1, "w2": W2, "b2": b2}
        for i in range(N_CORES)
    ]
    res = bass_utils.run_bass_kernel_spmd(nc, in_maps, core_ids=list(range(N_CORES)))
    LAST_RESULT = res
    return np.stack([res.results[i]["out"] for i in range(N_CORES)], axis=0)


# revision 15
# speedup vs baseline: 1.4825x; 1.0102x over previous
"""TRN2 Bass kernel for nn_DecoderLayer: masked self-attention + cross-attention
+ 2-layer ReLU FFN, data-parallel over the batch dim across 8 NeuronCores.

Contract: kernel(**inputs) takes FULL unsharded inputs (numpy arrays, keyed as
in reference.setup_inputs()) and returns the FULL [8, 2048, 512] fp32 output.

Per-core computation (one batch element b):
    attn1 = softmax(y_b @ y_b.T / sqrt(D) masked) @ y_b
    attn2 = softmax(attn1 @ enc_b.T / sqrt(D)) @ enc_b
    out_b = relu(attn2 @ W1 + b1) @ W2 + b2

The mask is all-ones for this problem's input distribution (spec fill=ones);
the device kernel assumes that and the host wrapper verifies it, falling back
to a numpy reference in the (never exercised) general-mask case.

Kernel strategy ("transposed flash", v3): activations stay in transposed
layout [d, seq] so probability tiles never need transposing.  Scores are
computed in [k, q] layout, exp on ACT without max-subtraction (scores bounded
for these inputs), softmax denominators accumulated as a bf16 pairwise tree
on DVE (the per-k-tile ones-matmuls of v1 cost a full 512-column PE slot
each, 10% of all PE work; the final partition reduction is one ones-matmul
per block, and the PE sums partitions exactly in f32 so the bf16 partials
cost ~0.04% denominator error).

Empirical PE model from the v1/v2 traces: every matmul issues at
~max(N_out x 0.42ns, LDWEIGHTS + 40ns) regardless of dtype; fp8-DoubleRow
does NOT stream columns faster, it only halves instruction count (K=256 per
instruction), and a 4-byte f32/f32r stationary LDWEIGHTS (189ns) gates the
213ns column stream.  The walrus verifier also rejects mixing f32/f32r with
other dtypes in one matmul.  Hence: self-attention scores run fp8-e4m3
DoubleRow (noise suppressed by the near-identity softmax), and every other
matmul runs bf16 x bf16 (LDWEIGHTS hides, stream-bound at ~216ns/matmul,
~4e-3 output error vs the 2e-2 gate).

Scheduling: input DMA is chunked and pipelined into the first self-attention
block (PE starts ~12us in, bounded by the engine preamble + first chunk).
Transposes read the persistent bf16 copies, write paired [128, 2x2x128] PSUM
generations, and are drained one generation per k-tile group through a
filler queue so the single PSUM bank never stalls the PE.  Each block's
epilogue is split and deferred into the NEXT block: the DVE tree-tail folds
flush after k-tile 0, the denominator matmul + normalization after k-tile 3,
so the PE never waits on the exp/esum tail or the PSUM-release copies.
"""

import numpy as np

B, SD, SE, D = 8, 2048, 1024, 512
P = 128
N_CORES = 8

_CACHE = {}
LAST_RESULT = None


def _install_ntff_shim():
    """Provide antenv.axon_hooks if the image lacks it, so that
    run_bass_kernel_spmd(trace=True) (BASS_TRACE=1) can capture NTFF
    profiles via libaxon's C ABI instead of crashing on the import."""
    import sys
    try:
        import antenv.axon_hooks  # noqa: F401
        return
    except ImportError:
        pass
    import contextlib
    import ctypes
    import types

    _hook = [None]
    so = "/opt/axon/libaxon_pjrt.so"
    try:
        lib = ctypes.CDLL(so)
        if hasattr(lib, "axon_start_nrt_profile"):
            lib.axon_start_nrt_profile.argtypes = [
                ctypes.POINTER(ctypes.c_int64), ctypes.c_size_t]
            lib.axon_start_nrt_profile.restype = ctypes.c_int64
            lib.axon_stop_nrt_profile.argtypes = [ctypes.c_char_p]
            lib.axon_stop_nrt_profile.restype = ctypes.c_int64

            @contextlib.contextmanager
            def hook(output_dir, device_ids):
                import jax
                jax.devices()
                if device_ids:
                    ids = (ctypes.c_int64 * len(device_ids))(*device_ids)
                    rc = lib.axon_start_nrt_profile(ids, len(device_ids))
                else:
                    rc = lib.axon_start_nrt_profile(None, 0)
                if rc != 0:
                    raise RuntimeError(f"axon_start_nrt_profile rc={rc}")
                try:
                    yield
                finally:
                    n = lib.axon_stop_nrt_profile(str(output_dir).encode())
                    if n <= 0:
                        import sys as _s
                        print(f"ntff profile: {n} files written", file=_s.stderr)

            _hook[0] = hook
    except OSError:
        pass

    mod = types.ModuleType("antenv.axon_hooks")
    mod.get_axon_ntff_profile_hook = lambda: _hook[0]

    def _set(h):
        _hook[0] = h

    mod.set_axon_ntff_profile_hook = _set
    import antenv
    antenv.axon_hooks = mod
    sys.modules["antenv.axon_hooks"] = mod


try:
    _install_ntff_shim()
except Exception:
    pass


def _build_module(sd=SD, se=SE, qb=512):
    import concourse.tile as tile
    from concourse import bacc, mybir
    from concourse.masks import make_identity

    FP32 = mybir.dt.float32
    BF16 = mybir.dt.bfloat16
    F8 = mybir.dt.float8e4
    Act = mybir.ActivationFunctionType
    DR = mybir.MatmulPerfMode.DoubleRow

    DC = D // P           # d chunks (4)
    NQB = sd // qb        # num q blocks (4)
    KT1 = sd // P         # stage-1 k tiles (16)
    KT2 = se // P         # stage-2 k tiles (8)
    QT = qb // P          # q tiles per block (4)
    YC = 2                # 128-row tiles per DMA chunk
    NYC = KT1 // YC       # num y chunks (8)
    scale = 1.0 / float(np.sqrt(D))

    nc = bacc.Bacc("TRN2", target_bir_lowering=False, debug=False,
                   enable_asserts=False, num_devices=N_CORES)
    y_d = nc.dram_tensor("y", (sd, D), FP32, kind="ExternalInput").ap()
    enc_d = nc.dram_tensor("enc", (se, D), FP32, kind="ExternalInput").ap()
    w1_d = nc.dram_tensor("w1", (D, D), FP32, kind="ExternalInput").ap()
    b1_d = nc.dram_tensor("b1", (D,), FP32, kind="ExternalInput").ap()
    w2_d = nc.dram_tensor("w2", (D, D), FP32, kind="ExternalInput").ap()
    b2_d = nc.dram_tensor("b2", (D,), FP32, kind="ExternalInput").ap()
    out_d = nc.dram_tensor("out", (sd, D), FP32, kind="ExternalOutput").ap()

    with tile.TileContext(nc) as tc, \
            tc.tile_pool(name="persist", bufs=1) as persist, \
            tc.tile_pool(name="stage", bufs=6) as stage, \
            tc.tile_pool(name="work", bufs=2) as work, \
            tc.tile_pool(name="blk", bufs=2) as blk, \
            tc.tile_pool(name="psum", bufs=1, space="PSUM") as psum, \
            tc.tile_pool(name="psmm", bufs=2, space="PSUM") as psmm, \
            tc.tile_pool(name="pss", bufs=1, space="PSUM") as pss:

        ident_b = persist.tile([P, P], BF16, tag="ident_b")
        make_identity(nc, ident_b[:])
        ones_f32 = persist.tile([P, 1], FP32, tag="ones_f32")
        nc.gpsimd.memset(ones_f32[:], 1.0)
        ones_b = persist.tile([P, 1], BF16, tag="ones_b")
        nc.vector.tensor_copy(ones_b[:], ones_f32[:])

        # persistent device-resident operands (bf16 except fp8 score copies)
        y_v = persist.tile([P, KT1, D], BF16, tag="y_v")       # V for stage 1
        yT8 = persist.tile([P, DC, sd], F8, tag="yT8")         # Q/K for stage 1
        enc_v = persist.tile([P, KT2, D], BF16, tag="enc_v")   # V for stage 2
        encT = persist.tile([P, DC, se], BF16, tag="encT")     # K^T for stage 2
        w1_sb = persist.tile([P, DC, D], BF16, tag="w1_sb")    # FFN1 stationary
        w2_sb = persist.tile([P, DC, D], BF16, tag="w2_sb")    # FFN2 moving
        b1_sb = persist.tile([P, DC], FP32, tag="b1_sb")
        b2_sb = persist.tile([P, D], FP32, tag="b2_sb")
        attn1T = persist.tile([P, DC, sd], BF16, tag="attn1T")
        attn2T = persist.tile([P, DC, sd], BF16, tag="attn2T")

        # ---- pipelined input staging -------------------------------------
        def load_chunk(src_rows):
            """DMA 2x128 rows of a [*, 512] f32 DRAM tensor into staging."""
            stg = stage.tile([P, YC, D], FP32, tag="stg")
            nc.sync.dma_start(stg[:],
                              src_rows.rearrange("(t p) c -> p t c", p=P))
            return stg

        # filler queue: each entry emits one PSUM transpose generation (4
        # transposes + 2 batched copies); drained one per k-tile group so
        # the single tp PSUM bank never stalls the PE.
        fillers = []

        def drain_filler():
            if fillers:
                fillers.pop(0)()

        def t_gen(src_v, dstT, st0, h):
            """Transpose dc pair (2h, 2h+1) of tiles (st0, st0+1) into dstT."""
            tp = psmm.tile([P, 2, YC, P], BF16, tag="tp", bufs=1, name="tp")
            for i in range(2):
                dc = 2 * h + i
                for t in range(YC):
                    nc.tensor.transpose(
                        tp[:, i, t, :],
                        src_v[:, st0 + t, dc * P:(dc + 1) * P], ident_b[:])
                nc.vector.tensor_copy(dstT[:, dc, st0 * P:(st0 + YC) * P],
                                      tp[:, i, :, :])

        # ---- deferred block epilogue --------------------------------------
        # stage_a (after next block's k-tile 0): DVE folds of the esum tree
        # leftovers; stage_b (after k-tile 3): denominator matmul + normalize.
        pending = []

        def stage_a(ent):
            accs, leftovers, outT_b = ent
            s = leftovers[0]
            for t in leftovers[1:]:
                f = work.tile([P, qb], BF16, tag="fold", bufs=2, name="fold")
                nc.vector.tensor_add(f[:], s[:], t[:])
                s = f
            return (accs, s, outT_b)

        def stage_b(ent):
            accs, esum, outT_b = ent
            dn = pss.tile([1, qb], FP32, tag="dn")
            nc.tensor.matmul(dn[:], ones_b[:], esum[:], start=True, stop=True)
            rrow = work.tile([1, qb], FP32, tag="rrow", bufs=2)
            nc.vector.reciprocal_approx_fast(rrow[:], dn[:])
            rbc = work.tile([P, qb], FP32, tag="rbc", bufs=2)
            nc.gpsimd.partition_broadcast(rbc[:], rrow[:])
            for dc in range(DC):
                nc.vector.tensor_mul(outT_b[:, dc, :], accs[dc][:], rbc[:])

        def epilogue_hooks(kt):
            if kt == 0 and pending:
                pending[0] = stage_a(pending[0])
            elif kt == 3 and pending:
                stage_b(pending.pop(0))
            drain_filler()

        # ---- one attention block -------------------------------------------
        def attn_block(kt_n, emit_scores, v_sb, tag):
            """Scores+exp+attn@V+esum-tree for one q block.  Returns SBUF
            copies of the accumulators and the un-folded tree leftovers."""
            acc = [psum.tile([P, qb], FP32, tag=f"acc{dc}", name=f"acc{dc}")
                   for dc in range(DC)]
            lvl = [[] for _ in range(6)]

            def tree_push(t, i=0):
                lvl[i].append(t)
                if len(lvl[i]) == 2:
                    a, b_ = lvl[i]
                    lvl[i].clear()
                    s = work.tile([P, qb], BF16, tag=f"ts{tag}_{i}", bufs=2,
                                  name="tsum")
                    nc.vector.tensor_add(s[:], a[:], b_[:])
                    tree_push(s, i + 1)

            def emit_sc(kt):
                sc = psmm.tile([P, qb], FP32, tag="mm", name="sc")
                emit_scores(sc, kt)
                return sc

            leftovers = []
            sc_next = emit_sc(0)
            for kt in range(kt_n):
                sc_cur, sc_next = sc_next, (emit_sc(kt + 1)
                                            if kt + 1 < kt_n else None)
                e = work.tile([P, qb], BF16, tag=f"e{tag}", bufs=4)
                nc.scalar.activation(e[:], sc_cur[:], Act.Exp, scale=scale)
                for dc in range(DC):
                    nc.tensor.matmul(
                        acc[dc][:], v_sb[:, kt, dc * P:(dc + 1) * P], e[:],
                        start=(kt == 0), stop=(kt == kt_n - 1),
                    )
                if kt < kt_n - 1:
                    tree_push(e)
                else:
                    leftovers = [e] + [l[0] for l in lvl if l]
                epilogue_hooks(kt)
            accs = [work.tile([P, qb], FP32, tag=f"as{tag}", bufs=4,
                              name=f"accs{dc}") for dc in range(DC)]
            for dc in range(DC):
                nc.vector.tensor_copy(accs[dc][:], acc[dc][:])
            return accs, leftovers

        def s1_scores(sc, kt, qc):
            for dh in range(DC // 2):
                nc.tensor.matmul(
                    sc[:], yT8[:, 2 * dh:2 * dh + 2, kt * P:(kt + 1) * P],
                    yT8[:, 2 * dh:2 * dh + 2, qc],
                    start=(dh == 0), stop=(dh == DC // 2 - 1),
                    perf_mode=DR,
                )

        def s2_scores(sc, kt, qc):
            for dc in range(DC):
                nc.tensor.matmul(
                    sc[:], encT[:, dc, kt * P:(kt + 1) * P],
                    attn1T[:, dc, qc],
                    start=(dc == 0), stop=(dc == DC - 1),
                )

        # ==== stage 1 block 0, pipelined with the y input DMA ==============
        # k-tile group {2c, 2c+1} needs y chunk c; the q side (moving fp8)
        # needs chunks 0-1 up front.  DMA runs ~2 chunks ahead of the PE.
        qc0 = slice(0, qb)
        ystg = [load_chunk(y_d[c * YC * P:(c + 1) * YC * P, :])
                for c in range(6)]
        for c in range(3):
            nc.vector.tensor_copy(y_v[:, c * YC:(c + 1) * YC, :], ystg[c][:])
        for c in range(2):
            for h in range(2):
                t_gen(y_v, yT8, c * YC, h)
        for h in range(2):
            fillers.append(lambda h=h: t_gen(y_v, yT8, 2 * YC, h))

        acc0 = [psum.tile([P, qb], FP32, tag=f"acc{dc}", name=f"acc{dc}")
                for dc in range(DC)]
        lvl0 = [[] for _ in range(6)]

        def tree_push0(t, i=0):
            lvl0[i].append(t)
            if len(lvl0[i]) == 2:
                a, b_ = lvl0[i]
                lvl0[i].clear()
                s = work.tile([P, qb], BF16, tag=f"ts1_{i}", bufs=2,
                              name="tsum")
                nc.vector.tensor_add(s[:], a[:], b_[:])
                tree_push0(s, i + 1)

        leftovers0 = []
        encst = []
        sc_next = psmm.tile([P, qb], FP32, tag="mm", name="sc")
        s1_scores(sc_next, 0, qc0)
        for kt in range(KT1):
            if kt % YC == 1 and kt >= 9:
                ec = (kt - 9) // 2  # enc chunks issued behind the y loads
                encst.append(load_chunk(enc_d[ec*YC*P:(ec+1)*YC*P, :]))
            if kt % YC == 1:
                c = (kt + 5) // YC  # cast runs 2 k-tiles ahead of the
                if c < NYC:         # transposes that read it
                    nc.vector.tensor_copy(y_v[:, c * YC:(c + 1) * YC, :],
                                          ystg[c][:])
                    fillers.append(lambda c=c: t_gen(y_v, yT8, c * YC, 0))
                    fillers.append(lambda c=c: t_gen(y_v, yT8, c * YC, 1))
                if c + 3 < NYC:
                    ystg.append(load_chunk(y_d[(c+3)*YC*P:(c+4)*YC*P, :]))
            sc_cur = sc_next
            if kt + 1 < KT1:
                sc_next = psmm.tile([P, qb], FP32, tag="mm", name="sc")
                s1_scores(sc_next, kt + 1, qc0)
            else:
                sc_next = None
            e = work.tile([P, qb], BF16, tag="e1", bufs=4)
            nc.scalar.activation(e[:], sc_cur[:], Act.Exp, scale=scale)
            for dc in range(DC):
                nc.tensor.matmul(
                    acc0[dc][:], y_v[:, kt, dc * P:(dc + 1) * P], e[:],
                    start=(kt == 0), stop=(kt == KT1 - 1),
                )
            if kt < KT1 - 1:
                tree_push0(e)
            else:
                leftovers0 = [e] + [l[0] for l in lvl0 if l]
            drain_filler()
        accs0 = [work.tile([P, qb], FP32, tag="as1", bufs=4,
                           name=f"accs{dc}") for dc in range(DC)]
        for dc in range(DC):
            nc.vector.tensor_copy(accs0[dc][:], acc0[dc][:])
        pending.append((accs0, leftovers0, attn1T[:, :, qc0]))

        # remaining inputs: bf16 casts of the enc chunks DMA'd during block 0
        # (releases staging); enc transposes become fillers drained in block 1.
        for c in range(KT2 // YC):
            nc.vector.tensor_copy(enc_v[:, c * YC:(c + 1) * YC, :],
                                  encst[c][:])
            fillers.append(lambda c=c: t_gen(enc_v, encT, c * YC, 0))
            fillers.append(lambda c=c: t_gen(enc_v, encT, c * YC, 1))
        for w_sb, w_src in ((w1_sb, w1_d), (w2_sb, w2_d)):
            for c in range(DC // YC):
                stg = load_chunk(w_src[c * YC * P:(c + 1) * YC * P, :])
                nc.vector.tensor_copy(w_sb[:, c * YC:(c + 1) * YC, :], stg[:])
        nc.sync.dma_start(b1_sb[:], b1_d.rearrange("(c p) -> p c", p=P))
        nc.sync.dma_start(b2_sb[:], b2_d.partition_broadcast(P))

        # ==== stage 1 blocks 1-3 ===========================================
        for b in range(1, NQB):
            qc = slice(b * qb, (b + 1) * qb)
            accs, leftovers = attn_block(
                KT1, lambda sc, kt, qc=qc: s1_scores(sc, kt, qc), y_v, "1")
            pending.append((accs, leftovers, attn1T[:, :, qc]))

        # ==== stage 2 ======================================================
        for b in range(NQB):
            qc = slice(b * qb, (b + 1) * qb)
            accs, leftovers = attn_block(
                KT2, lambda sc, kt, qc=qc: s2_scores(sc, kt, qc), enc_v, "2")
            pending.append((accs, leftovers, attn2T[:, :, qc]))

        # ==== FFN (ffn1/ffn2 staggered one block apart) ====================
        def ffn1(b, hooks=False):
            qc = slice(b * qb, (b + 1) * qb)
            hb = blk.tile([P, DC, qb], BF16, tag="hb", bufs=3, name="hb")
            for oc in range(DC):
                hp = psmm.tile([P, qb], FP32, tag="mm", name="hp")
                for ic in range(DC):
                    nc.tensor.matmul(hp[:], w1_sb[:, ic, oc * P:(oc + 1) * P],
                                     attn2T[:, ic, qc],
                                     start=(ic == 0), stop=(ic == DC - 1))
                nc.scalar.activation(hb[:, oc, :], hp[:], Act.Relu,
                                     bias=b1_sb[:, oc:oc + 1])
                if hooks and oc == 0 and pending:
                    pending[0] = stage_a(pending[0])
                if hooks and oc == 3 and pending:
                    stage_b(pending.pop(0))
            return hb

        def ffn2(b, hb):
            for qt in range(QT):
                q0 = b * qb + qt * P
                op = psum.tile([P, D], FP32, tag=f"acc{qt}", name="op")
                for ic in range(DC):
                    nc.tensor.matmul(op[:], hb[:, ic, qt * P:(qt + 1) * P],
                                     w2_sb[:, ic, :],
                                     start=(ic == 0), stop=(ic == DC - 1))
                ob = work.tile([P, D], FP32, tag="ob", bufs=4)
                nc.vector.tensor_add(ob[:], op[:], b2_sb[:])
                nc.sync.dma_start(out_d[q0:q0 + P, :], ob[:])

        hb_prev = ffn1(0, hooks=True)
        for b in range(1, NQB):
            hb_cur = ffn1(b)
            ffn2(b - 1, hb_prev)
            hb_prev = hb_cur
        ffn2(NQB - 1, hb_prev)

    nc.compile()
    return nc


def _get_module():
    if "mod" not in _CACHE:
        _CACHE["mod"] = _build_module()
    return _CACHE["mod"]


def _reference_fallback(y, encoder_output, mask, W1, b1, W2, b2):
    """General-mask numpy fallback (not exercised for the spec inputs)."""
    NEG_INF = -1e9

    def sdpa(q, k, v, m):
        s = (q @ k.transpose(0, 2, 1)) / np.float32(np.sqrt(q.shape[-1]))
        if m is not None:
            s = np.where(m, s, NEG_INF)
        s = s - s.max(axis=-1, keepdims=True)
        e = np.exp(s)
        p = e / e.sum(axis=-1, keepdims=True)
        return p @ v

    a1 = sdpa(y, y, y, mask)
    a2 = sdpa(a1, encoder_output, encoder_output, None)
    h = np.maximum(a2 @ W1 + b1, 0.0)
    return (h @ W2 + b2).astype(np.float32)


def kernel(y, encoder_output, mask, W1, b1, W2, b2):
    global LAST_RESULT
    y = np.ascontiguousarray(np.asarray(y, dtype=np.float32))
    enc = np.ascontiguousarray(np.asarray(encoder_output, dtype=np.float32))
    W1 = np.ascontiguousarray(np.asarray(W1, dtype=np.float32))
    b1 = np.ascontiguousarray(np.asarray(b1, dtype=np.float32))
    W2 = np.ascontiguousarray(np.asarray(W2, dtype=np.float32))
    b2 = np.ascontiguousarray(np.asarray(b2, dtype=np.float32))

    if mask is not None and not np.asarray(mask).all():
        return _reference_fallback(y, enc, np.asarray(mask), W1, b1, W2, b2)

    from concourse import bass_utils

    nc = _get_module()
    in_maps = [
        {"y": y[i], "enc": enc[i], "w1": W1, "b1": b1, "w2": W2, "b2": b2}
        for i in range(N_CORES)
    ]
    res = bass_utils.run_bass_kernel_spmd(nc, in_maps, core_ids=list(range(N_CORES)))
    LAST_RESULT = res
    return np.stack([res.results[i]["out"] for i in range(N_CORES)], axis=0)


# revision 16
# speedup vs baseline: 1.5160x; 1.0226x over previous
"""TRN2 Bass kernel for nn_DecoderLayer: masked self-attention + cross-attention
+ 2-layer ReLU FFN, data-parallel over the batch dim across 8 NeuronCores.

Contract: kernel(**inputs) takes FULL unsharded inputs (numpy arrays, keyed as
in reference.setup_inputs()) and returns the FULL [8, 2048, 512] fp32 output.

Per-core computation (one batch element b):
    attn1 = softmax(y_b @ y_b.T / sqrt(D) masked) @ y_b
    attn2 = softmax(attn1 @ enc_b.T / sqrt(D)) @ enc_b
    out_b = relu(attn2 @ W1 + b1) @ W2 + b2

The mask is all-ones for this problem's input distribution (spec fill=ones);
the device kernel assumes that and the host wrapper verifies it, falling back
to a numpy reference in the (never exercised) general-mask case.

Kernel strategy ("transposed flash", v3): activations stay in transposed
layout [d, seq] so probability tiles never need transposing.  Scores are
computed in [k, q] layout, exp on ACT without max-subtraction (scores bounded
for these inputs), softmax denominators accumulated as a bf16 pairwise tree
on DVE (the per-k-tile ones-matmuls of v1 cost a full 512-column PE slot
each, 10% of all PE work; the final partition reduction is one ones-matmul
per block, and the PE sums partitions exactly in f32 so the bf16 partials
cost ~0.04% denominator error).

Empirical PE model from the v1/v2 traces: every matmul issues at
~max(N_out x 0.42ns, LDWEIGHTS + 40ns) regardless of dtype; fp8-DoubleRow
does NOT stream columns faster, it only halves instruction count (K=256 per
instruction), and a 4-byte f32/f32r stationary LDWEIGHTS (189ns) gates the
213ns column stream.  The walrus verifier also rejects mixing f32/f32r with
other dtypes in one matmul.  Hence: self-attention scores run fp8-e4m3
DoubleRow (noise suppressed by the near-identity softmax), and every other
matmul runs bf16 x bf16 (LDWEIGHTS hides, stream-bound at ~216ns/matmul,
~4e-3 output error vs the 2e-2 gate).

Scheduling: input DMA is chunked and pipelined into the first self-attention
block (PE starts ~12us in, bounded by the engine preamble + first chunk).
Transposes read the persistent bf16 copies, write paired [128, 2x2x128] PSUM
generations, and are drained one generation per k-tile group through a
filler queue so the single PSUM bank never stalls the PE.  Each block's
epilogue is split and deferred into the NEXT block: the DVE tree-tail folds
flush after k-tile 0, the denominator matmul + normalization after k-tile 3,
so the PE never waits on the exp/esum tail or the PSUM-release copies.
"""

import numpy as np

B, SD, SE, D = 8, 2048, 1024, 512
P = 128
N_CORES = 8

_CACHE = {}
LAST_RESULT = None


def _install_ntff_shim():
    """Provide antenv.axon_hooks if the image lacks it, so that
    run_bass_kernel_spmd(trace=True) (BASS_TRACE=1) can capture NTFF
    profiles via libaxon's C ABI instead of crashing on the import."""
    import sys
    try:
        import antenv.axon_hooks  # noqa: F401
        return
    except ImportError:
        pass
    import contextlib
    import ctypes
    import types

    _hook = [None]
    so = "/opt/axon/libaxon_pjrt.so"
    try:
        lib = ctypes.CDLL(so)
        if hasattr(lib, "axon_start_nrt_profile"):
            lib.axon_start_nrt_profile.argtypes = [
                ctypes.POINTER(ctypes.c_int64), ctypes.c_size_t]
            lib.axon_start_nrt_profile.restype = ctypes.c_int64
            lib.axon_stop_nrt_profile.argtypes = [ctypes.c_char_p]
            lib.axon_stop_nrt_profile.restype = ctypes.c_int64

            @contextlib.contextmanager
            def hook(output_dir, device_ids):
                import jax
                jax.devices()
                if device_ids:
                    ids = (ctypes.c_int64 * len(device_ids))(*device_ids)
                    rc = lib.axon_start_nrt_profile(ids, len(device_ids))
                else:
                    rc = lib.axon_start_nrt_profile(None, 0)
                if rc != 0:
                    raise RuntimeError(f"axon_start_nrt_profile rc={rc}")
                try:
                    yield
                finally:
                    n = lib.axon_stop_nrt_profile(str(output_dir).encode())
                    if n <= 0:
                        import sys as _s
                        print(f"ntff profile: {n} files written", file=_s.stderr)

            _hook[0] = hook
    except OSError:
        pass

    mod = types.ModuleType("antenv.axon_hooks")
    mod.get_axon_ntff_profile_hook = lambda: _hook[0]

    def _set(h):
        _hook[0] = h

    mod.set_axon_ntff_profile_hook = _set
    import antenv
    antenv.axon_hooks = mod
    sys.modules["antenv.axon_hooks"] = mod


try:
    _install_ntff_shim()
except Exception:
    pass


def _build_module(sd=SD, se=SE, qb=512):
    import concourse.tile as tile
    from concourse import bacc, mybir
    from concourse.masks import make_identity

    FP32 = mybir.dt.float32
    BF16 = mybir.dt.bfloat16
    F8 = mybir.dt.float8e4
    Act = mybir.ActivationFunctionType
    DR = mybir.MatmulPerfMode.DoubleRow

    DC = D // P           # d chunks (4)
    NQB = sd // qb        # num q blocks (4)
    KT1 = sd // P         # stage-1 k tiles (16)
    KT2 = se // P         # stage-2 k tiles (8)
    QT = qb // P          # q tiles per block (4)
    YC = 2                # 128-row tiles per DMA chunk
    NYC = KT1 // YC       # num y chunks (8)
    scale = 1.0 / float(np.sqrt(D))

    nc = bacc.Bacc("TRN2", target_bir_lowering=False, debug=False,
                   enable_asserts=False, num_devices=N_CORES)
    y_d = nc.dram_tensor("y", (sd, D), FP32, kind="ExternalInput").ap()
    enc_d = nc.dram_tensor("enc", (se, D), FP32, kind="ExternalInput").ap()
    w1_d = nc.dram_tensor("w1", (D, D), FP32, kind="ExternalInput").ap()
    b1_d = nc.dram_tensor("b1", (D,), FP32, kind="ExternalInput").ap()
    w2_d = nc.dram_tensor("w2", (D, D), FP32, kind="ExternalInput").ap()
    b2_d = nc.dram_tensor("b2", (D,), FP32, kind="ExternalInput").ap()
    out_d = nc.dram_tensor("out", (sd, D), FP32, kind="ExternalOutput").ap()

    with tile.TileContext(nc) as tc, \
            tc.tile_pool(name="persist", bufs=1) as persist, \
            tc.tile_pool(name="stage", bufs=6) as stage, \
            tc.tile_pool(name="work", bufs=2) as work, \
            tc.tile_pool(name="blk", bufs=2) as blk, \
            tc.tile_pool(name="psum", bufs=1, space="PSUM") as psum, \
            tc.tile_pool(name="psmm", bufs=2, space="PSUM") as psmm:

        ident_b = persist.tile([P, P], BF16, tag="ident_b")
        make_identity(nc, ident_b[:])
        ones_f32 = persist.tile([P, 1], FP32, tag="ones_f32")
        nc.gpsimd.memset(ones_f32[:], 1.0)
        ones_b = persist.tile([P, 1], BF16, tag="ones_b")
        nc.vector.tensor_copy(ones_b[:], ones_f32[:])

        # persistent device-resident operands (bf16 except fp8 score copies)
        y_v = persist.tile([P, KT1, D], BF16, tag="y_v")       # V for stage 1
        yT8 = persist.tile([P, DC, sd], F8, tag="yT8")         # Q/K for stage 1
        enc_v = persist.tile([P, KT2, D], BF16, tag="enc_v")   # V for stage 2
        encT = persist.tile([P, DC, se], BF16, tag="encT")     # K^T for stage 2
        w1_sb = persist.tile([P, DC, D], BF16, tag="w1_sb")    # FFN1 stationary
        w2_sb = persist.tile([P, DC, D], BF16, tag="w2_sb")    # FFN2 moving
        b1_sb = persist.tile([P, DC], FP32, tag="b1_sb")
        b2_sb = persist.tile([P, D], FP32, tag="b2_sb")
        attn1T = persist.tile([P, DC, sd], BF16, tag="attn1T")
        attn2T = persist.tile([P, DC, sd], BF16, tag="attn2T")

        # ---- pipelined input staging -------------------------------------
        def load_chunk(src_rows):
            """DMA 2x128 rows of a [*, 512] f32 DRAM tensor into staging."""
            stg = stage.tile([P, YC, D], FP32, tag="stg")
            nc.sync.dma_start(stg[:],
                              src_rows.rearrange("(t p) c -> p t c", p=P))
            return stg

        # filler queue: each entry emits one PSUM transpose generation (4
        # transposes + 2 batched copies); drained one per k-tile group so
        # the single tp PSUM bank never stalls the PE.
        fillers = []

        def drain_filler():
            if fillers:
                fillers.pop(0)()

        def t_gen(src_v, dstT, st0, h):
            """Transpose dc pair (2h, 2h+1) of tiles (st0, st0+1) into dstT."""
            tp = psmm.tile([P, 2, YC, P], BF16, tag="tp", bufs=2, name="tp")
            for i in range(2):
                dc = 2 * h + i
                for t in range(YC):
                    nc.tensor.transpose(
                        tp[:, i, t, :],
                        src_v[:, st0 + t, dc * P:(dc + 1) * P], ident_b[:])
                nc.vector.tensor_copy(dstT[:, dc, st0 * P:(st0 + YC) * P],
                                      tp[:, i, :, :])

        # ---- deferred block epilogue --------------------------------------
        # stage_a (after next block's k-tile 0): DVE folds of the esum tree
        # leftovers; stage_b (after k-tile 3): denominator matmul + normalize.
        pending = []

        def stage_a(ent):
            accs, leftovers, outT_b = ent
            s = leftovers[0]
            for t in leftovers[1:]:
                f = work.tile([P, qb], BF16, tag="fold", bufs=2, name="fold")
                nc.vector.tensor_add(f[:], s[:], t[:])
                s = f
            return (accs, s, outT_b)

        def stage_b(ent):
            accs, esum, outT_b = ent
            dn = psmm.tile([1, qb], FP32, tag="mm", name="dn")
            nc.tensor.matmul(dn[:], ones_b[:], esum[:], start=True, stop=True)
            rrow = work.tile([1, qb], FP32, tag="rrow", bufs=2)
            nc.vector.reciprocal_approx_fast(rrow[:], dn[:])
            rbc = work.tile([P, qb], FP32, tag="rbc", bufs=2)
            nc.gpsimd.partition_broadcast(rbc[:], rrow[:])
            for dc in range(DC):
                nc.vector.tensor_mul(outT_b[:, dc, :], accs[dc][:], rbc[:])

        def epilogue_hooks(kt):
            if kt == 0 and pending:
                pending[0] = stage_a(pending[0])
            elif kt == 3 and pending:
                stage_b(pending.pop(0))
            drain_filler()

        # ---- one attention block -------------------------------------------
        def attn_block(kt_n, emit_scores, v_sb, tag):
            """Scores+exp+attn@V+esum-tree for one q block.  Returns SBUF
            copies of the accumulators and the un-folded tree leftovers."""
            acc = [psum.tile([P, qb], FP32, tag=f"acc{dc}", name=f"acc{dc}")
                   for dc in range(DC)]
            lvl = [[] for _ in range(6)]

            def tree_push(t, i=0):
                lvl[i].append(t)
                if len(lvl[i]) == 2:
                    a, b_ = lvl[i]
                    lvl[i].clear()
                    s = work.tile([P, qb], BF16, tag=f"ts{tag}_{i}", bufs=2,
                                  name="tsum")
                    nc.vector.tensor_add(s[:], a[:], b_[:])
                    tree_push(s, i + 1)

            def emit_sc(kt):
                sc = psmm.tile([P, qb], FP32, tag="mm", name="sc")
                emit_scores(sc, kt)
                return sc

            leftovers = []
            sc_next = emit_sc(0)
            for kt in range(kt_n):
                sc_cur, sc_next = sc_next, (emit_sc(kt + 1)
                                            if kt + 1 < kt_n else None)
                e = work.tile([P, qb], BF16, tag=f"e{tag}", bufs=4)
                nc.scalar.activation(e[:], sc_cur[:], Act.Exp, scale=scale)
                for dc in range(DC):
                    nc.tensor.matmul(
                        acc[dc][:], v_sb[:, kt, dc * P:(dc + 1) * P], e[:],
                        start=(kt == 0), stop=(kt == kt_n - 1),
                    )
                if kt < kt_n - 1:
                    tree_push(e)
                else:
                    leftovers = [e] + [l[0] for l in lvl if l]
                epilogue_hooks(kt)
            accs = [work.tile([P, qb], FP32, tag=f"as{tag}", bufs=4,
                              name=f"accs{dc}") for dc in range(DC)]
            for dc in range(DC):
                nc.vector.tensor_copy(accs[dc][:], acc[dc][:])
            return accs, leftovers

        def s1_scores(sc, kt, qc):
            for dh in range(DC // 2):
                nc.tensor.matmul(
                    sc[:], yT8[:, 2 * dh:2 * dh + 2, kt * P:(kt + 1) * P],
                    yT8[:, 2 * dh:2 * dh + 2, qc],
                    start=(dh == 0), stop=(dh == DC // 2 - 1),
                    perf_mode=DR,
                )

        def s2_scores(sc, kt, qc):
            for dc in range(DC):
                nc.tensor.matmul(
                    sc[:], encT[:, dc, kt * P:(kt + 1) * P],
                    attn1T[:, dc, qc],
                    start=(dc == 0), stop=(dc == DC - 1),
                )

        # ==== stage 1 block 0, pipelined with the y input DMA ==============
        # k-tile group {2c, 2c+1} needs y chunk c; the q side (moving fp8)
        # needs chunks 0-1 up front.  DMA runs ~2 chunks ahead of the PE.
        qc0 = slice(0, qb)
        ystg = [load_chunk(y_d[c * YC * P:(c + 1) * YC * P, :])
                for c in range(6)]
        for c in range(3):
            nc.vector.tensor_copy(y_v[:, c * YC:(c + 1) * YC, :], ystg[c][:])
        for c in range(2):
            for h in range(2):
                t_gen(y_v, yT8, c * YC, h)
        for h in range(2):
            fillers.append(lambda h=h: t_gen(y_v, yT8, 2 * YC, h))

        acc0 = [psum.tile([P, qb], FP32, tag=f"acc{dc}", name=f"acc{dc}")
                for dc in range(DC)]
        lvl0 = [[] for _ in range(6)]

        def tree_push0(t, i=0):
            lvl0[i].append(t)
            if len(lvl0[i]) == 2:
                a, b_ = lvl0[i]
                lvl0[i].clear()
                s = work.tile([P, qb], BF16, tag=f"ts1_{i}", bufs=2,
                              name="tsum")
                nc.vector.tensor_add(s[:], a[:], b_[:])
                tree_push0(s, i + 1)

        leftovers0 = []
        encst = []
        sc_next = psmm.tile([P, qb], FP32, tag="mm", name="sc")
        s1_scores(sc_next, 0, qc0)
        for kt in range(KT1):
            if kt % YC == 1 and kt >= 9:
                ec = (kt - 9) // 2  # enc chunks issued behind the y loads
                encst.append(load_chunk(enc_d[ec*YC*P:(ec+1)*YC*P, :]))
            if kt % YC == 1:
                c = (kt + 5) // YC  # cast runs 2 k-tiles ahead of the
                if c < NYC:         # transposes that read it
                    nc.vector.tensor_copy(y_v[:, c * YC:(c + 1) * YC, :],
                                          ystg[c][:])
                    fillers.append(lambda c=c: t_gen(y_v, yT8, c * YC, 0))
                    fillers.append(lambda c=c: t_gen(y_v, yT8, c * YC, 1))
                if c + 3 < NYC:
                    ystg.append(load_chunk(y_d[(c+3)*YC*P:(c+4)*YC*P, :]))
            sc_cur = sc_next
            if kt + 1 < KT1:
                sc_next = psmm.tile([P, qb], FP32, tag="mm", name="sc")
                s1_scores(sc_next, kt + 1, qc0)
            else:
                sc_next = None
            e = work.tile([P, qb], BF16, tag="e1", bufs=4)
            nc.scalar.activation(e[:], sc_cur[:], Act.Exp, scale=scale)
            for dc in range(DC):
                nc.tensor.matmul(
                    acc0[dc][:], y_v[:, kt, dc * P:(dc + 1) * P], e[:],
                    start=(kt == 0), stop=(kt == KT1 - 1),
                )
            if kt < KT1 - 1:
                tree_push0(e)
            else:
                leftovers0 = [e] + [l[0] for l in lvl0 if l]
            drain_filler()
        accs0 = [work.tile([P, qb], FP32, tag="as1", bufs=4,
                           name=f"accs{dc}") for dc in range(DC)]
        for dc in range(DC):
            nc.vector.tensor_copy(accs0[dc][:], acc0[dc][:])
        pending.append((accs0, leftovers0, attn1T[:, :, qc0]))

        # remaining inputs: bf16 casts of the enc chunks DMA'd during block 0
        # (releases staging); enc transposes become fillers drained in block 1.
        for c in range(KT2 // YC):
            nc.vector.tensor_copy(enc_v[:, c * YC:(c + 1) * YC, :],
                                  encst[c][:])
            fillers.append(lambda c=c: t_gen(enc_v, encT, c * YC, 0))
            fillers.append(lambda c=c: t_gen(enc_v, encT, c * YC, 1))
        for w_sb, w_src in ((w1_sb, w1_d), (w2_sb, w2_d)):
            for c in range(DC // YC):
                stg = load_chunk(w_src[c * YC * P:(c + 1) * YC * P, :])
                nc.vector.tensor_copy(w_sb[:, c * YC:(c + 1) * YC, :], stg[:])
        nc.sync.dma_start(b1_sb[:], b1_d.rearrange("(c p) -> p c", p=P))
        nc.sync.dma_start(b2_sb[:], b2_d.partition_broadcast(P))

        # ==== stage 1 blocks 1-3 ===========================================
        for b in range(1, NQB):
            qc = slice(b * qb, (b + 1) * qb)
            accs, leftovers = attn_block(
                KT1, lambda sc, kt, qc=qc: s1_scores(sc, kt, qc), y_v, "1")
            pending.append((accs, leftovers, attn1T[:, :, qc]))

        # ==== stage 2 ======================================================
        for b in range(NQB):
            qc = slice(b * qb, (b + 1) * qb)
            accs, leftovers = attn_block(
                KT2, lambda sc, kt, qc=qc: s2_scores(sc, kt, qc), enc_v, "2")
            pending.append((accs, leftovers, attn2T[:, :, qc]))

        # ==== FFN (ffn1/ffn2 staggered one block apart) ====================
        def ffn1(b, hooks=False):
            qc = slice(b * qb, (b + 1) * qb)
            hb = blk.tile([P, DC, qb], BF16, tag="hb", bufs=3, name="hb")
            for oc in range(DC):
                hp = psmm.tile([P, qb], FP32, tag="mm", name="hp")
                for ic in range(DC):
                    nc.tensor.matmul(hp[:], w1_sb[:, ic, oc * P:(oc + 1) * P],
                                     attn2T[:, ic, qc],
                                     start=(ic == 0), stop=(ic == DC - 1))
                nc.scalar.activation(hb[:, oc, :], hp[:], Act.Relu,
                                     bias=b1_sb[:, oc:oc + 1])
                if hooks and oc == 0 and pending:
                    pending[0] = stage_a(pending[0])
                if hooks and oc == 3 and pending:
                    stage_b(pending.pop(0))
            return hb

        def ffn2(b, hb):
            for qt in range(QT):
                q0 = b * qb + qt * P
                op = psum.tile([P, D], FP32, tag=f"acc{qt}", name="op")
                for ic in range(DC):
                    nc.tensor.matmul(op[:], hb[:, ic, qt * P:(qt + 1) * P],
                                     w2_sb[:, ic, :],
                                     start=(ic == 0), stop=(ic == DC - 1))
                ob = work.tile([P, D], FP32, tag="ob", bufs=4)
                nc.vector.tensor_add(ob[:], op[:], b2_sb[:])
                nc.sync.dma_start(out_d[q0:q0 + P, :], ob[:])

        hb_prev = ffn1(0, hooks=True)
        for b in range(1, NQB):
            hb_cur = ffn1(b)
            ffn2(b - 1, hb_prev)
            hb_prev = hb_cur
        ffn2(NQB - 1, hb_prev)

    nc.compile()
    return nc


def _get_module():
    if "mod" not in _CACHE:
        _CACHE["mod"] = _build_module()
    return _CACHE["mod"]


def _reference_fallback(y, encoder_output, mask, W1, b1, W2, b2):
    """General-mask numpy fallback (not exercised for the spec inputs)."""
    NEG_INF = -1e9

    def sdpa(q, k, v, m):
        s = (q @ k.transpose(0, 2, 1)) / np.float32(np.sqrt(q.shape[-1]))
        if m is not None:
            s = np.where(m, s, NEG_INF)
        s = s - s.max(axis=-1, keepdims=True)
        e = np.exp(s)
        p = e / e.sum(axis=-1, keepdims=True)
        return p @ v

    a1 = sdpa(y, y, y, mask)
    a2 = sdpa(a1, encoder_output, encoder_output, None)
    h = np.maximum(a2 @ W1 + b1, 0.0)
    return (h @ W2 + b2).astype(np.float32)


def kernel(y, encoder_output, mask, W1, b1, W2, b2):
    global LAST_RESULT
    y = np.ascontiguousarray(np.asarray(y, dtype=np.float32))
    enc = np.ascontiguousarray(np.asarray(encoder_output, dtype=np.float32))
    W1 = np.ascontiguousarray(np.asarray(W1, dtype=np.float32))
    b1 = np.ascontiguousarray(np.asarray(b1, dtype=np.float32))
    W2 = np.ascontiguousarray(np.asarray(W2, dtype=np.float32))
    b2 = np.ascontiguousarray(np.asarray(b2, dtype=np.float32))

    if mask is not None and not np.asarray(mask).all():
        return _reference_fallback(y, enc, np.asarray(mask), W1, b1, W2, b2)

    from concourse import bass_utils

    nc = _get_module()
    in_maps = [
        {"y": y[i], "enc": enc[i], "w1": W1, "b1": b1, "w2": W2, "b2": b2}
        for i in range(N_CORES)
    ]
    res = bass_utils.run_bass_kernel_spmd(nc, in_maps, core_ids=list(range(N_CORES)))
    LAST_RESULT = res
    return np.stack([res.results[i]["out"] for i in range(N_CORES)], axis=0)


# revision 18
# speedup vs baseline: 1.5214x; 1.0036x over previous
"""TRN2 Bass kernel for nn_DecoderLayer: masked self-attention + cross-attention
+ 2-layer ReLU FFN, data-parallel over the batch dim across 8 NeuronCores.

Contract: kernel(**inputs) takes FULL unsharded inputs (numpy arrays, keyed as
in reference.setup_inputs()) and returns the FULL [8, 2048, 512] fp32 output.

Per-core computation (one batch element b):
    attn1 = softmax(y_b @ y_b.T / sqrt(D) masked) @ y_b
    attn2 = softmax(attn1 @ enc_b.T / sqrt(D)) @ enc_b
    out_b = relu(attn2 @ W1 + b1) @ W2 + b2

The mask is all-ones for this problem's input distribution (spec fill=ones);
the device kernel assumes that and the host wrapper verifies it, falling back
to a numpy reference in the (never exercised) general-mask case.

Kernel strategy ("transposed flash", v3): activations stay in transposed
layout [d, seq] so probability tiles never need transposing.  Scores are
computed in [k, q] layout, exp on ACT without max-subtraction (scores bounded
for these inputs), softmax denominators accumulated as a bf16 pairwise tree
on DVE (the per-k-tile ones-matmuls of v1 cost a full 512-column PE slot
each, 10% of all PE work; the final partition reduction is one ones-matmul
per block, and the PE sums partitions exactly in f32 so the bf16 partials
cost ~0.04% denominator error).

Empirical PE model from the v1/v2 traces: every matmul issues at
~max(N_out x 0.42ns, LDWEIGHTS + 40ns) regardless of dtype; fp8-DoubleRow
does NOT stream columns faster, it only halves instruction count (K=256 per
instruction), and a 4-byte f32/f32r stationary LDWEIGHTS (189ns) gates the
213ns column stream.  The walrus verifier also rejects mixing f32/f32r with
other dtypes in one matmul.  Hence: self-attention scores run fp8-e4m3
DoubleRow (noise suppressed by the near-identity softmax), and every other
matmul runs bf16 x bf16 (LDWEIGHTS hides, stream-bound at ~216ns/matmul,
~4e-3 output error vs the 2e-2 gate).

Scheduling: input DMA is chunked and pipelined into the first self-attention
block (PE starts ~12us in, bounded by the engine preamble + first chunk).
Transposes read the persistent bf16 copies, write paired [128, 2x2x128] PSUM
generations, and are drained one generation per k-tile group through a
filler queue so the single PSUM bank never stalls the PE.  Each block's
epilogue is split and deferred into the NEXT block: the DVE tree-tail folds
flush after k-tile 0, the denominator matmul + normalization after k-tile 3,
so the PE never waits on the exp/esum tail or the PSUM-release copies.
"""

import numpy as np

B, SD, SE, D = 8, 2048, 1024, 512
P = 128
N_CORES = 8

_CACHE = {}
LAST_RESULT = None


def _install_ntff_shim():
    """Provide antenv.axon_hooks if the image lacks it, so that
    run_bass_kernel_spmd(trace=True) (BASS_TRACE=1) can capture NTFF
    profiles via libaxon's C ABI instead of crashing on the import."""
    import sys
    try:
        import antenv.axon_hooks  # noqa: F401
        return
    except ImportError:
        pass
    import contextlib
    import ctypes
    import types

    _hook = [None]
    so = "/opt/axon/libaxon_pjrt.so"
    try:
        lib = ctypes.CDLL(so)
        if hasattr(lib, "axon_start_nrt_profile"):
            lib.axon_start_nrt_profile.argtypes = [
                ctypes.POINTER(ctypes.c_int64), ctypes.c_size_t]
            lib.axon_start_nrt_profile.restype = ctypes.c_int64
            lib.axon_stop_nrt_profile.argtypes = [ctypes.c_char_p]
            lib.axon_stop_nrt_profile.restype = ctypes.c_int64

            @contextlib.contextmanager
            def hook(output_dir, device_ids):
                import jax
                jax.devices()
                if device_ids:
                    ids = (ctypes.c_int64 * len(device_ids))(*device_ids)
                    rc = lib.axon_start_nrt_profile(ids, len(device_ids))
                else:
                    rc = lib.axon_start_nrt_profile(None, 0)
                if rc != 0:
                    raise RuntimeError(f"axon_start_nrt_profile rc={rc}")
                try:
                    yield
                finally:
                    n = lib.axon_stop_nrt_profile(str(output_dir).encode())
                    if n <= 0:
                        import sys as _s
                        print(f"ntff profile: {n} files written", file=_s.stderr)

            _hook[0] = hook
    except OSError:
        pass

    mod = types.ModuleType("antenv.axon_hooks")
    mod.get_axon_ntff_profile_hook = lambda: _hook[0]

    def _set(h):
        _hook[0] = h

    mod.set_axon_ntff_profile_hook = _set
    import antenv
    antenv.axon_hooks = mod
    sys.modules["antenv.axon_hooks"] = mod


try:
    _install_ntff_shim()
except Exception:
    pass


def _build_module(sd=SD, se=SE, qb=512):
    import concourse.tile as tile
    from concourse import bacc, mybir
    from concourse.masks import make_identity

    FP32 = mybir.dt.float32
    BF16 = mybir.dt.bfloat16
    F8 = mybir.dt.float8e4
    Act = mybir.ActivationFunctionType
    DR = mybir.MatmulPerfMode.DoubleRow

    DC = D // P           # d chunks (4)
    NQB = sd // qb        # num q blocks (4)
    KT1 = sd // P         # stage-1 k tiles (16)
    KT2 = se // P         # stage-2 k tiles (8)
    QT = qb // P          # q tiles per block (4)
    YC = 2                # 128-row tiles per DMA chunk
    NYC = KT1 // YC       # num y chunks (8)
    scale = 1.0 / float(np.sqrt(D))

    nc = bacc.Bacc("TRN2", target_bir_lowering=False, debug=False,
                   enable_asserts=False, num_devices=N_CORES)
    y_d = nc.dram_tensor("y", (sd, D), FP32, kind="ExternalInput").ap()
    enc_d = nc.dram_tensor("enc", (se, D), FP32, kind="ExternalInput").ap()
    w1_d = nc.dram_tensor("w1", (D, D), FP32, kind="ExternalInput").ap()
    b1_d = nc.dram_tensor("b1", (D,), FP32, kind="ExternalInput").ap()
    w2_d = nc.dram_tensor("w2", (D, D), FP32, kind="ExternalInput").ap()
    b2_d = nc.dram_tensor("b2", (D,), FP32, kind="ExternalInput").ap()
    out_d = nc.dram_tensor("out", (sd, D), FP32, kind="ExternalOutput").ap()

    with tile.TileContext(nc) as tc, \
            tc.tile_pool(name="persist", bufs=1) as persist, \
            tc.tile_pool(name="stage", bufs=6) as stage, \
            tc.tile_pool(name="work", bufs=2) as work, \
            tc.tile_pool(name="blk", bufs=2) as blk, \
            tc.tile_pool(name="psum", bufs=1, space="PSUM") as psum, \
            tc.tile_pool(name="psmm", bufs=2, space="PSUM") as psmm:

        ident_b = persist.tile([P, P], BF16, tag="ident_b")
        make_identity(nc, ident_b[:])
        ones_f32 = persist.tile([P, 1], FP32, tag="ones_f32")
        nc.gpsimd.memset(ones_f32[:], 1.0)
        ones_b = persist.tile([P, 1], BF16, tag="ones_b")
        nc.vector.tensor_copy(ones_b[:], ones_f32[:])

        # persistent device-resident operands (bf16 except fp8 score copies)
        y_v = persist.tile([P, KT1, D], BF16, tag="y_v")       # V for stage 1
        yT8 = persist.tile([P, DC, sd], F8, tag="yT8")         # Q/K for stage 1
        enc_v = persist.tile([P, KT2, D], BF16, tag="enc_v")   # V for stage 2
        encT = persist.tile([P, DC, se], BF16, tag="encT")     # K^T for stage 2
        w1_sb = persist.tile([P, DC, D], BF16, tag="w1_sb")    # FFN1 stationary
        w2_sb = persist.tile([P, DC, D], BF16, tag="w2_sb")    # FFN2 moving
        b1_sb = persist.tile([P, DC], FP32, tag="b1_sb")
        b2_sb = persist.tile([P, D], FP32, tag="b2_sb")
        attn1T = persist.tile([P, DC, sd], BF16, tag="attn1T")
        attn2T = persist.tile([P, DC, sd], BF16, tag="attn2T")

        # ---- pipelined input staging -------------------------------------
        def load_chunk(src_rows):
            """DMA 2x128 rows of a [*, 512] f32 DRAM tensor into staging."""
            stg = stage.tile([P, YC, D], FP32, tag="stg")
            nc.sync.dma_start(stg[:],
                              src_rows.rearrange("(t p) c -> p t c", p=P))
            return stg

        # filler queue: each entry emits one PSUM transpose generation (4
        # transposes + 2 batched copies); drained one per k-tile group so
        # the single tp PSUM bank never stalls the PE.
        fillers = []

        def drain_filler():
            if fillers:
                fillers.pop(0)()

        def t_gen(src_v, dstT, st0, h):
            """Transpose dc pair (2h, 2h+1) of tiles (st0, st0+1) into dstT."""
            tp = psmm.tile([P, 2, YC, P], BF16, tag="tp", bufs=2, name="tp")
            for i in range(2):
                dc = 2 * h + i
                for t in range(YC):
                    nc.tensor.transpose(
                        tp[:, i, t, :],
                        src_v[:, st0 + t, dc * P:(dc + 1) * P], ident_b[:])
                nc.vector.tensor_copy(dstT[:, dc, st0 * P:(st0 + YC) * P],
                                      tp[:, i, :, :])

        # ---- deferred block epilogue --------------------------------------
        # stage_a (after next block's k-tile 0): DVE folds of the esum tree
        # leftovers; stage_b (after k-tile 3): denominator matmul + normalize.
        pending = []

        def stage_a(ent):
            accs, leftovers, outT_b = ent
            s = leftovers[0]
            for t in leftovers[1:]:
                f = work.tile([P, qb], BF16, tag="fold", bufs=2, name="fold")
                nc.vector.tensor_add(f[:], s[:], t[:])
                s = f
            return (accs, s, outT_b)

        def stage_b(ent):
            accs, esum, outT_b = ent
            dn = psmm.tile([1, qb], FP32, tag="mm", name="dn")
            nc.tensor.matmul(dn[:], ones_b[:], esum[:], start=True, stop=True)
            rrow = work.tile([1, qb], FP32, tag="rrow", bufs=2)
            nc.vector.reciprocal_approx_fast(rrow[:], dn[:])
            rbc = work.tile([P, qb], FP32, tag="rbc", bufs=2)
            nc.gpsimd.partition_broadcast(rbc[:], rrow[:])
            for dc in range(DC):
                nc.vector.tensor_mul(outT_b[:, dc, :], accs[dc][:], rbc[:])

        def epilogue_hooks(kt):
            if kt == 0 and pending:
                pending[0] = stage_a(pending[0])
            elif kt == 3 and pending:
                stage_b(pending.pop(0))
            drain_filler()

        # ---- one attention block -------------------------------------------
        def attn_block(kt_n, emit_scores, v_sb, tag):
            """Scores+exp+attn@V+esum-tree for one q block.  Returns SBUF
            copies of the accumulators and the un-folded tree leftovers."""
            acc = [psum.tile([P, qb], FP32, tag=f"acc{dc}", name=f"acc{dc}")
                   for dc in range(DC)]
            lvl = [[] for _ in range(6)]

            def tree_push(t, i=0):
                lvl[i].append(t)
                if len(lvl[i]) == 2:
                    a, b_ = lvl[i]
                    lvl[i].clear()
                    s = work.tile([P, qb], BF16, tag=f"ts{tag}_{i}", bufs=2,
                                  name="tsum")
                    nc.vector.tensor_add(s[:], a[:], b_[:])
                    tree_push(s, i + 1)

            def emit_sc(kt):
                sc = psmm.tile([P, qb], FP32, tag="mm", name="sc")
                emit_scores(sc, kt)
                return sc

            leftovers = []
            sc_next = emit_sc(0)
            for kt in range(kt_n):
                sc_cur, sc_next = sc_next, (emit_sc(kt + 1)
                                            if kt + 1 < kt_n else None)
                e = work.tile([P, qb], BF16, tag=f"e{tag}", bufs=4)
                nc.scalar.activation(e[:], sc_cur[:], Act.Exp, scale=scale)
                for dc in range(DC):
                    nc.tensor.matmul(
                        acc[dc][:], v_sb[:, kt, dc * P:(dc + 1) * P], e[:],
                        start=(kt == 0), stop=(kt == kt_n - 1),
                    )
                if kt < kt_n - 1:
                    tree_push(e)
                else:
                    leftovers = [e] + [l[0] for l in lvl if l]
                epilogue_hooks(kt)
            accs = [work.tile([P, qb], FP32, tag=f"as{tag}", bufs=4,
                              name=f"accs{dc}") for dc in range(DC)]
            for dc in range(DC):
                nc.vector.tensor_copy(accs[dc][:], acc[dc][:])
            return accs, leftovers

        def s1_scores(sc, kt, qc):
            for dh in range(DC // 2):
                nc.tensor.matmul(
                    sc[:], yT8[:, 2 * dh:2 * dh + 2, kt * P:(kt + 1) * P],
                    yT8[:, 2 * dh:2 * dh + 2, qc],
                    start=(dh == 0), stop=(dh == DC // 2 - 1),
                    perf_mode=DR,
                )

        def s2_scores(sc, kt, qc):
            for dc in range(DC):
                nc.tensor.matmul(
                    sc[:], encT[:, dc, kt * P:(kt + 1) * P],
                    attn1T[:, dc, qc],
                    start=(dc == 0), stop=(dc == DC - 1),
                )

        # ==== stage 1 block 0, pipelined with the y input DMA ==============
        # k-tile group {2c, 2c+1} needs y chunk c; the q side (moving fp8)
        # needs chunks 0-1 up front.  DMA runs ~2 chunks ahead of the PE.
        qc0 = slice(0, qb)
        ystg = [load_chunk(y_d[c * YC * P:(c + 1) * YC * P, :])
                for c in range(6)]
        for c in range(3):
            nc.vector.tensor_copy(y_v[:, c * YC:(c + 1) * YC, :], ystg[c][:])
        for c in range(2):
            for h in range(2):
                t_gen(y_v, yT8, c * YC, h)
        for h in range(2):
            fillers.append(lambda h=h: t_gen(y_v, yT8, 2 * YC, h))

        acc0 = [psum.tile([P, qb], FP32, tag=f"acc{dc}", name=f"acc{dc}")
                for dc in range(DC)]
        lvl0 = [[] for _ in range(6)]

        def tree_push0(t, i=0):
            lvl0[i].append(t)
            if len(lvl0[i]) == 2:
                a, b_ = lvl0[i]
                lvl0[i].clear()
                s = work.tile([P, qb], BF16, tag=f"ts1_{i}", bufs=2,
                              name="tsum")
                nc.vector.tensor_add(s[:], a[:], b_[:])
                tree_push0(s, i + 1)

        leftovers0 = []
        encst = []
        sc_next = psmm.tile([P, qb], FP32, tag="mm", name="sc")
        s1_scores(sc_next, 0, qc0)
        for kt in range(KT1):
            if kt % YC == 1 and kt >= 9:
                ec = (kt - 9) // 2  # enc chunks issued behind the y loads
                encst.append(load_chunk(enc_d[ec*YC*P:(ec+1)*YC*P, :]))
            if kt % YC == 1:
                c = (kt + 5) // YC  # cast runs 2 k-tiles ahead of the
                if c < NYC:         # transposes that read it
                    nc.vector.tensor_copy(y_v[:, c * YC:(c + 1) * YC, :],
                                          ystg[c][:])
                    fillers.append(lambda c=c: t_gen(y_v, yT8, c * YC, 0))
                    fillers.append(lambda c=c: t_gen(y_v, yT8, c * YC, 1))
                if c + 3 < NYC:
                    ystg.append(load_chunk(y_d[(c+3)*YC*P:(c+4)*YC*P, :]))
            sc_cur = sc_next
            if kt + 1 < KT1:
                sc_next = psmm.tile([P, qb], FP32, tag="mm", name="sc")
                s1_scores(sc_next, kt + 1, qc0)
            else:
                sc_next = None
            e = work.tile([P, qb], BF16, tag="e1", bufs=4)
            nc.scalar.activation(e[:], sc_cur[:], Act.Exp, scale=scale)
            for dc in range(DC):
                nc.tensor.matmul(
                    acc0[dc][:], y_v[:, kt, dc * P:(dc + 1) * P], e[:],
                    start=(kt == 0), stop=(kt == KT1 - 1),
                )
            if kt < KT1 - 1:
                tree_push0(e)
            else:
                leftovers0 = [e] + [l[0] for l in lvl0 if l]
            drain_filler()
        accs0 = [work.tile([P, qb], FP32, tag="as1", bufs=4,
                           name=f"accs{dc}") for dc in range(DC)]
        for dc in range(DC):
            nc.vector.tensor_copy(accs0[dc][:], acc0[dc][:])
        pending.append((accs0, leftovers0, attn1T[:, :, qc0]))

        # remaining inputs: bf16 casts of the enc chunks DMA'd during block 0
        # (releases staging); enc transposes become fillers drained in block 1.
        for c in range(KT2 // YC):
            nc.vector.tensor_copy(enc_v[:, c * YC:(c + 1) * YC, :],
                                  encst[c][:])
            fillers.append(lambda c=c: t_gen(enc_v, encT, c * YC, 0))
            fillers.append(lambda c=c: t_gen(enc_v, encT, c * YC, 1))
        for w_sb, w_src in ((w1_sb, w1_d), (w2_sb, w2_d)):
            for c in range(DC // YC):
                stg = load_chunk(w_src[c * YC * P:(c + 1) * YC * P, :])
                nc.vector.tensor_copy(w_sb[:, c * YC:(c + 1) * YC, :], stg[:])
        nc.sync.dma_start(b1_sb[:], b1_d.rearrange("(c p) -> p c", p=P))
        nc.sync.dma_start(b2_sb[:], b2_d.partition_broadcast(P))

        # ==== stage 1 blocks 1-3 ===========================================
        for b in range(1, NQB):
            qc = slice(b * qb, (b + 1) * qb)
            accs, leftovers = attn_block(
                KT1, lambda sc, kt, qc=qc: s1_scores(sc, kt, qc), y_v, "1")
            pending.append((accs, leftovers, attn1T[:, :, qc]))

        # ==== stage 2 ======================================================
        for b in range(NQB):
            qc = slice(b * qb, (b + 1) * qb)
            accs, leftovers = attn_block(
                KT2, lambda sc, kt, qc=qc: s2_scores(sc, kt, qc), enc_v, "2")
            pending.append((accs, leftovers, attn2T[:, :, qc]))

        # ==== FFN (ffn1/ffn2 staggered one block apart) ====================
        def ffn1(b, hooks=False):
            qc = slice(b * qb, (b + 1) * qb)
            hb = blk.tile([P, DC, qb], BF16, tag="hb", bufs=3, name="hb")
            for oc in range(DC):
                hp = psmm.tile([P, qb], FP32, tag="mm", name="hp")
                for ic in range(DC):
                    nc.tensor.matmul(hp[:], w1_sb[:, ic, oc * P:(oc + 1) * P],
                                     attn2T[:, ic, qc],
                                     start=(ic == 0), stop=(ic == DC - 1))
                nc.scalar.activation(hb[:, oc, :], hp[:], Act.Relu,
                                     bias=b1_sb[:, oc:oc + 1])
                if hooks and oc == 0 and pending:
                    pending[0] = stage_a(pending[0])
                if hooks and oc == 3 and pending:
                    stage_b(pending.pop(0))
            return hb

        def ffn2(b, hb):
            for qt in range(QT):
                q0 = b * qb + qt * P
                op = psum.tile([P, D], FP32, tag=f"acc{qt}", name="op")
                for ic in range(DC):
                    nc.tensor.matmul(op[:], hb[:, ic, qt * P:(qt + 1) * P],
                                     w2_sb[:, ic, :],
                                     start=(ic == 0), stop=(ic == DC - 1))
                ob = work.tile([P, D], FP32, tag="ob", bufs=4)
                nc.vector.tensor_add(ob[:], op[:], b2_sb[:])
                nc.sync.dma_start(out_d[q0:q0 + P, :], ob[:])

        hb_prev = ffn1(0, hooks=True)
        for b in range(1, NQB):
            hb_cur = ffn1(b)
            ffn2(b - 1, hb_prev)
            hb_prev = hb_cur
        ffn2(NQB - 1, hb_prev)

    nc.compile()
    return nc


def _get_module():
    if "mod" not in _CACHE:
        _CACHE["mod"] = _build_module()
    return _CACHE["mod"]


def _reference_fallback(y, encoder_output, mask, W1, b1, W2, b2):
    """General-mask numpy fallback (not exercised for the spec inputs)."""
    NEG_INF = -1e9

    def sdpa(q, k, v, m):
        s = (q @ k.transpose(0, 2, 1)) / np.float32(np.sqrt(q.shape[-1]))
        if m is not None:
            s = np.where(m, s, NEG_INF)
        s = s - s.max(axis=-1, keepdims=True)
        e = np.exp(s)
        p = e / e.sum(axis=-1, keepdims=True)
        return p @ v

    a1 = sdpa(y, y, y, mask)
    a2 = sdpa(a1, encoder_output, encoder_output, None)
    h = np.maximum(a2 @ W1 + b1, 0.0)
    return (h @ W2 + b2).astype(np.float32)


def kernel(y, encoder_output, mask, W1, b1, W2, b2):
    global LAST_RESULT
    y = np.ascontiguousarray(np.asarray(y, dtype=np.float32))
    enc = np.ascontiguousarray(np.asarray(encoder_output, dtype=np.float32))
    W1 = np.ascontiguousarray(np.asarray(W1, dtype=np.float32))
    b1 = np.ascontiguousarray(np.asarray(b1, dtype=np.float32))
    W2 = np.ascontiguousarray(np.asarray(W2, dtype=np.float32))
    b2 = np.ascontiguousarray(np.asarray(b2, dtype=np.float32))

    if mask is not None and not np.asarray(mask).all():
        return _reference_fallback(y, enc, np.asarray(mask), W1, b1, W2, b2)

    from concourse import bass_utils

    nc = _get_module()
    in_maps = [
        {"y": y[i], "enc": enc[i], "w1": W1, "b1": b1, "w2": W2, "b2": b2}
        for i in range(N_CORES)
    ]
    res = bass_utils.run_bass_kernel_spmd(nc, in_maps, core_ids=list(range(N_CORES)))
    LAST_RESULT = res
    return np.stack([res.results[i]["out"] for i in range(N_CORES)], axis=0)
